# revision 1
# baseline (speedup 1.0000x reference)
"""Trainium2 Bass kernel for EnhancedGNNTransformerEncoder (GNN message passing).

Sharding: dst-node sharding across 8 NeuronCores. Per-core node permutation
(degree- and A-count-sorted, padded to 6272 rows/core) makes every node-table
access affine. Per layer, each core computes k|v rows for its own shard on the
PE and an AllGather builds the replicated k|v table; per-edge k|v rows are then
fetched with two batched dma_gather calls per 128-dst group (table split at row
31360 so int16 indices fit). Edge layout: dst nodes on partitions, in-edges as
near-uniform rounds, so segment softmax/aggregation are strided DVE ops. Layer
outputs are transposed on the PE and kept feature-major in SBUF, feeding the
next layer's matmuls without DMA transposes.
"""

import os
import sys
import time

sys.path.insert(0, "/opt/trn_rl_repo")

import numpy as np

N = 50000
E = 800000
D = 128
H = 8
C = 16
L = 4
OUT = 128
NCORES = 8
PERCORE = N // NCORES       # 6250
PN = 6272                   # padded per-core rows (multiple of 128)
NPAD = PN * NCORES          # 50176
G = PN // 128               # 49 groups per core
SPLIT = 5 * PN              # 31360: k|v table rows < SPLIT use gather A

_cache = {}


def _host_prep(edge_index):
    src = np.asarray(edge_index[0]).astype(np.int64)
    dst = np.asarray(edge_index[1]).astype(np.int64)
    deg = np.bincount(dst, minlength=N)
    # A-edge iff src's core <= 4 iff src < 5*PERCORE (position block < SPLIT)
    a_cnt = np.bincount(dst[src < 5 * PERCORE], minlength=N)

    pi = np.full(NPAD, -1, dtype=np.int64)
    pos_of = np.zeros(N, dtype=np.int64)
    for c in range(NCORES):
        own = np.arange(c * PERCORE, (c + 1) * PERCORE)
        # sort by the binding normalized dimension max(A/5, B/3) desc (A
        # covers 5/8 of sources, B 3/8), then by A: keeps per-group max(A)
        # and max(B) jointly tight and aligns profiles across cores (round
        # counts are global maxima over all 8 cores)
        b_cnt = deg[own] - a_cnt[own]
        binding = np.maximum(a_cnt[own] / 5.0, b_cnt / 3.0)
        order = own[np.lexsort((-(a_cnt[own] / 5.0 - b_cnt / 3.0), -binding))]
        pi[c * PN: c * PN + PERCORE] = order
        pos_of[order] = c * PN + np.arange(PERCORE)

    # per-(core, group, lane) A/B counts -> global RA/RB per group
    lane_a = np.zeros((NCORES, PN), dtype=np.int64)
    lane_b = np.zeros((NCORES, PN), dtype=np.int64)
    for c in range(NCORES):
        nodes = pi[c * PN: c * PN + PERCORE]
        lane_a[c, :PERCORE] = a_cnt[nodes]
        lane_b[c, :PERCORE] = deg[nodes] - a_cnt[nodes]
    RA = np.zeros(G, dtype=np.int64)
    RB = np.zeros(G, dtype=np.int64)
    for g in range(G):
        RA[g] = lane_a[:, g * 128:(g + 1) * 128].max()
        RB[g] = lane_b[:, g * 128:(g + 1) * 128].max()
    RB += (RA + RB) % 2          # even round count per group
    Rg = RA + RB

    # reorder the 49 groups (blocks of 128 lanes) so equal-R groups are
    # adjacent: the kernel then processes several same-R groups in one set
    # of wide DVE ops. A/B membership only depends on the source core, so
    # permuting whole blocks within a core is free.
    perm = np.argsort(-Rg, kind="stable")
    RA, RB, Rg = RA[perm], RB[perm], Rg[perm]
    pi2 = np.full(NPAD, -1, dtype=np.int64)
    for c in range(NCORES):
        blocks = pi[c * PN:(c + 1) * PN].reshape(G, 128)
        pi2[c * PN:(c + 1) * PN] = blocks[perm].reshape(-1)
    pi = pi2
    for c in range(NCORES):
        own_mask = pi[c * PN:(c + 1) * PN] >= 0
        pos_of[pi[c * PN:(c + 1) * PN][own_mask]] = \
            c * PN + np.nonzero(own_mask)[0]

    # batches of consecutive equal-R groups, capped at RCAP total rounds
    RCAP = 48
    batches = []
    g = 0
    while g < G:
        R = int(Rg[g])
        k = 1
        while (g + k < G and int(Rg[g + k]) == R and R > 0
               and (k + 1) * R <= RCAP):
            k += 1
        batches.append((g, k, R))
        g += k

    base = np.concatenate([[0], np.cumsum(Rg)]).astype(np.int64)
    R_tot = int(base[-1])

    order_e = np.lexsort((src, dst))
    src_s = src[order_e]
    starts = np.concatenate([[0], np.cumsum(deg)]).astype(np.int64)

    kv_idx = np.zeros((NCORES, 16, 8 * R_tot), dtype=np.int16)
    mask = np.full((NCORES, 128, R_tot), -60000.0, dtype=np.float16)
    for c in range(NCORES):
        for g in range(G):
            ra, rb, b0 = int(RA[g]), int(RB[g]), int(base[g])
            for p in range(128):
                lane = g * 128 + p
                node = pi[c * PN + lane]
                if node < 0:
                    continue
                d_n = int(deg[node])
                if d_n == 0:
                    continue
                positions = pos_of[src_s[starts[node]: starts[node] + d_n]]
                pa = positions[positions < SPLIT]
                pb = positions[positions >= SPLIT] - SPLIT
                # A block: slots i = r*128+p at [i%16, 8*b0 + i//16]
                for r, v in enumerate(pa):
                    i = r * 128 + p
                    kv_idx[c, i % 16, 8 * b0 + i // 16] = v
                for r, v in enumerate(pb):
                    i = r * 128 + p
                    kv_idx[c, i % 16, 8 * (b0 + ra) + i // 16] = v
                mask[c, p, b0: b0 + len(pa)] = 0.0
                mask[c, p, b0 + ra: b0 + ra + len(pb)] = 0.0
    kv_idx_full = np.tile(kv_idx, (1, 8, 1))   # replicate for 8 gpsimd cores
    return pi, RA, RB, R_tot, kv_idx_full, mask, batches


def _build_program(RA, RB, R_tot, batches, L_EFF=L):
    import concourse.bass as bass
    import concourse.mybir as mybir
    from concourse import bacc, masks
    from concourse.tile import TileContext

    fp16 = mybir.dt.float16
    fp32 = mybir.dt.float32
    i16 = mybir.dt.int16
    AX = mybir.AxisListType
    ALU = mybir.AluOpType
    ACTF = mybir.ActivationFunctionType

    nc = bacc.Bacc("TRN2", target_bir_lowering=False, debug=False,
                   num_devices=NCORES)

    xownT_d = nc.dram_tensor("xownT", [D, PN], fp16, kind="ExternalInput")
    wkv_d = nc.dram_tensor("wkv", [D, L * 2 * D], fp16, kind="ExternalInput")
    wqs_d = nc.dram_tensor("wqs", [D, L * 2 * D], fp16, kind="ExternalInput")
    wout_d = nc.dram_tensor("wout", [D, OUT], fp16, kind="ExternalInput")
    kvidx_d = nc.dram_tensor("kvidx", [128, 8 * R_tot], i16, kind="ExternalInput")
    mask_d = nc.dram_tensor("mask", [128, R_tot], fp16, kind="ExternalInput")
    y_d = nc.dram_tensor("y", [PN, OUT], fp32, kind="ExternalOutput")

    kvown = [nc.dram_tensor(f"kvown{l}", [PN, 2 * D], fp16, kind="Internal")
             for l in range(L_EFF)]
    kvtab = [nc.dram_tensor(f"kvtab{l}", [NPAD, 2 * D], fp16, kind="Internal",
                            addr_space="Shared") for l in range(L_EFF)]

    base = np.concatenate([[0], np.cumsum(RA + RB)]).astype(np.int64)

    with TileContext(nc) as tc:
        with (
            tc.tile_pool(name="persist", bufs=1) as pp,
            tc.tile_pool(name="rows", bufs=3) as tp,
            tc.tile_pool(name="edge", bufs=2) as ep,
            tc.tile_pool(name="small", bufs=2) as sp,
            tc.tile_pool(name="psum", bufs=4, space="PSUM") as psp,
            tc.tile_pool(name="psumT", bufs=2, space="PSUM") as pspT,
        ):
            wkv_s = pp.tile([128, L * 2 * D], fp16, tag="wkv")
            nc.sync.dma_start(wkv_s[:], wkv_d[:])
            wqs_s = pp.tile([128, L * 2 * D], fp16, tag="wqs")
            nc.sync.dma_start(wqs_s[:], wqs_d[:])
            wout_s = pp.tile([128, OUT], fp16, tag="wout")
            nc.sync.dma_start(wout_s[:], wout_d[:])
            kvidx_s = pp.tile([128, 8 * R_tot], i16, tag="kvidx")
            nc.sync.dma_start(kvidx_s[:], kvidx_d[:])
            mask_s = pp.tile([128, R_tot], fp16, tag="mask")
            nc.sync.dma_start(mask_s[:], mask_d[:])
            ident = pp.tile([128, 128], fp16, tag="ident")
            masks.make_identity(nc, ident[:])

            xTo = [pp.tile([128, PN], fp16, tag=f"xTo{i}", name=f"xTo{i}")
                   for i in range(2)]
            qs_all = [pp.tile([128, G * 2 * D], fp16, tag=f"qs{i}", name=f"qs{i}")
                      for i in range(2)]
            nc.sync.dma_start(xTo[0][:], xownT_d[:])

            for l in range(L_EFF):
                xT = xTo[l % 2]
                qs = qs_all[l % 2]
                # ---- own-shard k|v rows (feed the AllGather) ----
                for g in range(G):
                    ps = psp.tile([128, 2 * D], fp32, tag="ps")
                    nc.tensor.matmul(
                        ps[:], xT[:, g * 128:(g + 1) * 128],
                        wkv_s[:, l * 2 * D:(l + 1) * 2 * D])
                    row = tp.tile([128, 2 * D], fp16, tag="row")
                    nc.vector.tensor_copy(row[:], ps[:])
                    nc.sync.dma_start(kvown[l][g * 128:(g + 1) * 128, :], row[:])
                # ---- q|skip for own nodes (stays in SBUF) ----
                for g in range(G):
                    ps = psp.tile([128, 2 * D], fp32, tag="ps")
                    nc.tensor.matmul(
                        ps[:], xT[:, g * 128:(g + 1) * 128],
                        wqs_s[:, l * 2 * D:(l + 1) * 2 * D])
                    nc.vector.tensor_copy(
                        qs[:, g * 2 * D:(g + 1) * 2 * D], ps[:])

                if not os.environ.get("KB_NO_COLL"):
                    nc.gpsimd.collective_compute(
                        "AllGather", ALU.bypass,
                        replica_groups=[list(range(NCORES))],
                        ins=[kvown[l][:]], outs=[kvtab[l][:]])
                tc.strict_bb_all_engine_barrier()

                # ---- edge phase: batches of K same-R groups ----
                xTn = xTo[(l + 1) % 2]
                for (g0, K, R) in batches:
                    KR = K * R
                    b0 = int(base[g0])
                    qsb = qs[:, g0 * 2 * D:(g0 + K) * 2 * D]
                    xn = sp.tile([128, K * D], fp16, tag="xn")
                    if R == 0:
                        nc.vector.tensor_scalar_max(
                            xn[:].rearrange("p (k d) -> p k d", k=K),
                            qsb.rearrange("p (k t) -> p k t", k=K)
                            [:, :, D:2 * D], 0.0)
                    else:
                        kv = ep.tile([128, KR, 2 * D], fp16, tag="kv")
                        # SWDGE ring holds 1024 descriptors; one instruction
                        # must stay below that (7 rounds = 896)
                        CH = 7
                        for k in range(K):
                            ra, rb = int(RA[g0 + k]), int(RB[g0 + k])
                            bk = int(base[g0 + k])
                            for r0 in range(0, ra, CH):
                                rc = min(CH, ra - r0)
                                nc.gpsimd.dma_gather(
                                    kv[:, k * R + r0:k * R + r0 + rc, :],
                                    kvtab[l][:],
                                    kvidx_s[:, 8 * (bk + r0):
                                            8 * (bk + r0 + rc)],
                                    num_idxs=128 * rc, num_idxs_reg=128 * rc,
                                    elem_size=2 * D)
                            for r0 in range(0, rb, CH):
                                rc = min(CH, rb - r0)
                                nc.gpsimd.dma_gather(
                                    kv[:, k * R + ra + r0:
                                       k * R + ra + r0 + rc, :],
                                    kvtab[l][SPLIT:NPAD, :],
                                    kvidx_s[:, 8 * (bk + ra + r0):
                                            8 * (bk + ra + r0 + rc)],
                                    num_idxs=128 * rc, num_idxs_reg=128 * rc,
                                    elem_size=2 * D)
                        qk = ep.tile([128, KR, D], fp16, tag="qkmsg")
                        nc.vector.tensor_mul(
                            qk[:].rearrange("p (k r) d -> p k r d", k=K),
                            kv[:, :, 0:D].rearrange(
                                "p (k r) d -> p k r d", k=K),
                            qsb.rearrange("p (k t) -> p k t", k=K)[:, :, 0:D]
                            .unsqueeze(2).broadcast_to([128, K, R, D]))
                        qk4 = qk[:].rearrange("p kr (h c) -> p kr h c", h=H)
                        w = C
                        while w > 2:
                            hw = w // 2
                            nc.vector.tensor_tensor(
                                qk4[:, :, :, 0:hw], qk4[:, :, :, 0:hw],
                                qk4[:, :, :, hw:w], op=ALU.add)
                            w = hw
                        scm = sp.tile([128, KR * H], fp16, tag="scm")
                        nc.vector.tensor_tensor(
                            scm[:].rearrange("p (kr h) -> p kr h", h=H),
                            qk4[:, :, :, 0:1].rearrange(
                                "p kr h c -> p kr (h c)"),
                            qk4[:, :, :, 1:2].rearrange(
                                "p kr h c -> p kr (h c)"),
                            op=ALU.add)
                        nc.vector.tensor_tensor(
                            scm[:].rearrange("p (kr h) -> p kr h", h=H),
                            scm[:].rearrange("p (kr h) -> p kr h", h=H),
                            mask_s[:, b0:b0 + KR].unsqueeze(2)
                            .broadcast_to([128, KR, H]),
                            op=ALU.add)
                        mx = sp.tile([128, K * H], fp16, tag="mx")
                        nc.vector.reduce_max(
                            mx[:].rearrange("p (k h) -> p k h", k=K),
                            scm[:].rearrange("p (k r h) -> p k h r", k=K, h=H),
                            axis=AX.X)
                        pexp = sp.tile([128, KR * H], fp16, tag="pexp")
                        nc.vector.tensor_tensor(
                            pexp[:].rearrange("p (k r h) -> p k r h",
                                              k=K, h=H),
                            scm[:].rearrange("p (k r h) -> p k r h",
                                             k=K, h=H),
                            mx[:].rearrange("p (k h) -> p k h", k=K)
                            .unsqueeze(2).broadcast_to([128, K, R, H]),
                            op=ALU.subtract)
                        pexps = sp.tile([128, KR * H], fp16, tag="pexps")
                        nc.scalar.activation(pexps[:], pexp[:], ACTF.Exp)
                        pe128 = ep.tile([128, KR, D], fp16, tag="qkmsg")
                        nc.scalar.activation(
                            pe128[:].rearrange("p kr (h c) -> p kr h c", h=H),
                            pexp[:].rearrange("p (kr h) -> p kr h", h=H)
                            .unsqueeze(3).broadcast_to([128, KR, H, C]),
                            ACTF.Exp)
                        z = sp.tile([128, K * H], fp32, tag="z")
                        nc.vector.reduce_sum(
                            z[:].rearrange("p (k h) -> p k h", k=K),
                            pexps[:].rearrange("p (k r h) -> p k h r",
                                               k=K, h=H),
                            axis=AX.X)
                        zi = sp.tile([128, K * H], fp16, tag="zi")
                        with nc.allow_low_precision("alpha normalizer fp16"):
                            nc.vector.reciprocal(zi[:], z[:])
                        nc.vector.tensor_tensor(
                            pe128[:], kv[:, :, D:2 * D], pe128[:],
                            op=ALU.mult)
                        pe4 = pe128[:].rearrange("p (k r) d -> p k r d", k=K)
                        n = R
                        while n > 1:
                            hw = n // 2
                            nc.vector.tensor_tensor(
                                pe4[:, :, 0:hw, :], pe4[:, :, 0:hw, :],
                                pe4[:, :, hw:2 * hw, :], op=ALU.add)
                            if n % 2 == 1:
                                nc.vector.tensor_tensor(
                                    pe4[:, :, 0:1, :], pe4[:, :, 0:1, :],
                                    pe4[:, :, 2 * hw:2 * hw + 1, :],
                                    op=ALU.add)
                            n = hw
                        xs = sp.tile([128, K * D], fp16, tag="xs")
                        nc.vector.tensor_tensor(
                            xs[:].rearrange("p (k h c) -> p k h c",
                                            k=K, h=H),
                            pe4[:, :, 0:1, :].rearrange(
                                "p k r (h c) -> p k (r h) c", h=H),
                            zi[:].rearrange("p (k h) -> p k h", k=K)
                            .unsqueeze(3).broadcast_to([128, K, H, C]),
                            op=ALU.mult)
                        nc.vector.tensor_tensor(
                            xs[:].rearrange("p (k d) -> p k d", k=K),
                            xs[:].rearrange("p (k d) -> p k d", k=K),
                            qsb.rearrange("p (k t) -> p k t", k=K)
                            [:, :, D:2 * D],
                            op=ALU.add)
                        nc.vector.tensor_scalar_max(xn[:], xs[:], 0.0)
                    psT = pspT.tile([128, K * 128], fp16, tag="psT")
                    for k in range(K):
                        nc.tensor.matmul(
                            psT[:, k * 128:(k + 1) * 128],
                            xn[:, k * D:(k + 1) * D], ident[:],
                            is_transpose=True)
                    nc.vector.tensor_copy(
                        xTn[:, g0 * 128:(g0 + K) * 128], psT[:])

            # ---- final projection (own nodes) ----
            xT = xTo[L_EFF % 2]
            for g in range(G):
                ps = psp.tile([128, 2 * D], fp32, tag="ps")
                nc.tensor.matmul(ps[:, 0:OUT], xT[:, g * 128:(g + 1) * 128],
                                 wout_s[:])
                yo = tp.tile([128, OUT], fp32, tag="yo")
                nc.vector.tensor_copy(yo[:], ps[:, 0:OUT])
                nc.sync.dma_start(y_d[g * 128:(g + 1) * 128, :], yo[:])

    nc.compile()
    return nc


def _make_runner(nc, n_cores=NCORES):
    import jax
    from jax.sharding import Mesh, PartitionSpec
    from jax.experimental.shard_map import shard_map
    import concourse.mybir as mybir
    from concourse import bass2jax

    bass2jax.install_neuronx_cc_hook()
    partition_name = nc.partition_id_tensor.name if nc.partition_id_tensor else None
    in_names, out_names, out_avals, zero_outs = [], [], [], []
    for alloc in nc.m.functions[0].allocations:
        if not isinstance(alloc, mybir.MemoryLocationSet):
            continue
        name = alloc.memorylocations[0].name
        if alloc.kind == "ExternalInput":
            if name != partition_name:
                in_names.append(name)
        elif alloc.kind == "ExternalOutput":
            shape = tuple(alloc.tensor_shape)
            dtype = mybir.dt.np(alloc.dtype)
            out_names.append(name)
            out_avals.append(jax.core.ShapedArray(shape, dtype))
            zero_outs.append(np.zeros(shape, dtype))
    n_params = len(in_names)
    n_outs = len(out_avals)
    all_in_names = in_names + out_names + ([partition_name] if partition_name else [])
    donate = tuple(range(n_params, n_params + n_outs))

    def _body(*args):
        operands = list(args)
        if partition_name is not None:
            operands.append(bass2jax.partition_id_tensor())
        outs = bass2jax._bass_exec_p.bind(
            *operands, out_avals=tuple(out_avals), in_names=tuple(all_in_names),
            out_names=tuple(out_names), lowering_input_output_aliases=(),
            sim_require_finite=True, sim_require_nnan=True, nc=nc)
        return tuple(outs)

    devices = jax.devices()[:n_cores]
    mesh = Mesh(np.asarray(devices), ("core",))
    in_specs = (PartitionSpec("core"),) * (n_params + n_outs)
    out_specs = (PartitionSpec("core"),) * n_outs
    fn = jax.jit(shard_map(_body, mesh=mesh, in_specs=in_specs,
                           out_specs=out_specs, check_rep=False),
                 keep_unused=True)

    def run(in_maps, time_reps=0):
        concat_in = [
            np.concatenate([np.asarray(in_maps[c][nm]) for c in range(n_cores)], axis=0)
            for nm in in_names]
        concat_zeros = [np.zeros((n_cores * z.shape[0], *z.shape[1:]), z.dtype)
                        for z in zero_outs]
        args = [jax.device_put(a) for a in concat_in + concat_zeros]
        out = fn(*args)
        jax.block_until_ready(out)
        tmin = None
        if time_reps:
            ts = []
            for _ in range(time_reps):
                t0 = time.perf_counter()
                out = fn(*args)
                jax.block_until_ready(out)
                ts.append(time.perf_counter() - t0)
            tmin = min(ts)
        results = [
            {nm: np.asarray(out[i]).reshape(n_cores, *out_avals[i].shape)[c]
             for i, nm in enumerate(out_names)}
            for c in range(n_cores)]
        return results, tmin
    return run


def kernel(**inputs):
    x = np.asarray(inputs["x"], dtype=np.float32)
    edge_index = np.asarray(inputs["edge_index"])
    Wq = np.asarray(inputs["Wq"], dtype=np.float32)
    Wk = np.asarray(inputs["Wk"], dtype=np.float32)
    Wv = np.asarray(inputs["Wv"], dtype=np.float32)
    Wskip = np.asarray(inputs["Wskip"], dtype=np.float32)
    Wout = np.asarray(inputs["Wout"], dtype=np.float32)
    bout = np.asarray(inputs["bout"], dtype=np.float32)
    for b in ("bq", "bk", "bv", "bskip"):
        assert np.all(np.asarray(inputs[b]) == 0.0), f"{b} must be zero"

    if "prog" not in _cache:
        pi, RA, RB, R_tot, kv_idx, mask, batches = _host_prep(edge_index)
        nc = _build_program(RA, RB, R_tot, batches,
                            L_EFF=int(os.environ.get("KB_LAYERS", str(L))))
        run = _make_runner(nc)
        _cache["prog"] = (pi, R_tot, kv_idx, mask, run)
    pi, R_tot, kv_idx, mask, run = _cache["prog"]

    # q gets the 1/sqrt(C)=0.25 attention scale folded in
    wkv = np.transpose(np.concatenate([Wk, Wv], axis=2), (1, 0, 2)).reshape(
        D, L * 2 * D).astype(np.float16)
    wqs = np.transpose(np.concatenate([Wq * 0.25, Wskip], axis=2),
                       (1, 0, 2)).reshape(D, L * 2 * D).astype(np.float16)
    x0 = np.zeros((NPAD, D), dtype=np.float16)
    valid = pi >= 0
    x0[valid] = x[pi[valid]].astype(np.float16)
    in_maps = []
    for c in range(NCORES):
        in_maps.append({
            "xownT": np.ascontiguousarray(x0[c * PN:(c + 1) * PN].T),
            "wkv": wkv,
            "wqs": wqs,
            "wout": Wout.astype(np.float16),
            "kvidx": kv_idx[c],
            "mask": mask[c],
        })
    results, tmin = run(in_maps, time_reps=int(os.environ.get("KB_TIME_REPS", "0")))
    globals()['_last_results'] = results
    if tmin is not None:
        globals()['_last_wall_us'] = tmin * 1e6
        print(f"kernel exec wall (min): {tmin*1e6:.0f} us")

    out = np.zeros((N, OUT), dtype=np.float32)
    for c in range(NCORES):
        yc = results[c]["y"]
        ids = pi[c * PN:(c + 1) * PN]
        m = ids >= 0
        out[ids[m]] = yc[m]
    return out + bout[None, :]



# revision 12
# speedup vs baseline: 4.6690x; 4.6690x over previous
"""Trainium2 Bass kernel for EnhancedGNNTransformerEncoder (GNN message passing).

Sharding: dst-node sharding across 8 NeuronCores. Per-core node permutation
(degree- and A-count-sorted, padded to 6272 rows/core) makes every node-table
access affine. Per layer, each core computes k|v rows for its own shard on the
PE and an AllGather builds the replicated k|v table; per-edge k|v rows are then
fetched with two batched dma_gather calls per 128-dst group (table split at row
31360 so int16 indices fit). Edge layout: dst nodes on partitions, in-edges as
near-uniform rounds, so segment softmax/aggregation are strided DVE ops. Layer
outputs are transposed on the PE and kept feature-major in SBUF, feeding the
next layer's matmuls without DMA transposes.
"""

import os
import sys
import time

sys.path.insert(0, "/opt/trn_rl_repo")

import numpy as np

N = 50000
E = 800000
D = 128
H = 8
C = 16
L = 4
OUT = 128
NCORES = 8
PERCORE = N // NCORES       # 6250
PN = 6272                   # padded per-core rows (multiple of 128)
NPAD = PN * NCORES          # 50176
G = PN // 128               # 49 groups per core
SPLIT = 5 * PN              # 31360: k|v table rows < SPLIT use gather A

_cache = {}


def _host_prep(edge_index):
    src = np.asarray(edge_index[0]).astype(np.int64)
    dst = np.asarray(edge_index[1]).astype(np.int64)
    deg = np.bincount(dst, minlength=N)
    # A-edge iff src's core <= 4 iff src < 5*PERCORE (position block < SPLIT)
    a_cnt = np.bincount(dst[src < 5 * PERCORE], minlength=N)

    pi = np.full(NPAD, -1, dtype=np.int64)
    pos_of = np.zeros(N, dtype=np.int64)
    for c in range(NCORES):
        own = np.arange(c * PERCORE, (c + 1) * PERCORE)
        # sort by the binding normalized dimension max(A/5, B/3) desc (A
        # covers 5/8 of sources, B 3/8), then by A: keeps per-group max(A)
        # and max(B) jointly tight and aligns profiles across cores (round
        # counts are global maxima over all 8 cores)
        b_cnt = deg[own] - a_cnt[own]
        binding = np.maximum(a_cnt[own] / 5.0, b_cnt / 3.0)
        order = own[np.lexsort((-(a_cnt[own] / 5.0 - b_cnt / 3.0), -binding))]
        pi[c * PN: c * PN + PERCORE] = order
        pos_of[order] = c * PN + np.arange(PERCORE)

    # per-(core, group, lane) A/B counts -> global RA/RB per group
    lane_a = np.zeros((NCORES, PN), dtype=np.int64)
    lane_b = np.zeros((NCORES, PN), dtype=np.int64)
    for c in range(NCORES):
        nodes = pi[c * PN: c * PN + PERCORE]
        lane_a[c, :PERCORE] = a_cnt[nodes]
        lane_b[c, :PERCORE] = deg[nodes] - a_cnt[nodes]
    RA = np.zeros(G, dtype=np.int64)
    RB = np.zeros(G, dtype=np.int64)
    for g in range(G):
        RA[g] = lane_a[:, g * 128:(g + 1) * 128].max()
        RB[g] = lane_b[:, g * 128:(g + 1) * 128].max()
    RB += (RA + RB) % 2          # even round count per group
    Rg = RA + RB

    # reorder the 49 groups (blocks of 128 lanes) so equal-R groups are
    # adjacent: the kernel then processes several same-R groups in one set
    # of wide DVE ops. A/B membership only depends on the source core, so
    # permuting whole blocks within a core is free.
    perm = np.argsort(-Rg, kind="stable")
    RA, RB, Rg = RA[perm], RB[perm], Rg[perm]
    pi2 = np.full(NPAD, -1, dtype=np.int64)
    for c in range(NCORES):
        blocks = pi[c * PN:(c + 1) * PN].reshape(G, 128)
        pi2[c * PN:(c + 1) * PN] = blocks[perm].reshape(-1)
    pi = pi2
    for c in range(NCORES):
        own_mask = pi[c * PN:(c + 1) * PN] >= 0
        pos_of[pi[c * PN:(c + 1) * PN][own_mask]] = \
            c * PN + np.nonzero(own_mask)[0]

    # batches of consecutive equal-R groups, capped at RCAP total rounds
    RCAP = 48
    batches = []
    g = 0
    while g < G:
        R = int(Rg[g])
        k = 1
        while (g + k < G and int(Rg[g + k]) == R and R > 0
               and (k + 1) * R <= RCAP):
            k += 1
        batches.append((g, k, R))
        g += k

    base = np.concatenate([[0], np.cumsum(Rg)]).astype(np.int64)
    R_tot = int(base[-1])

    order_e = np.lexsort((src, dst))
    src_s = src[order_e]
    starts = np.concatenate([[0], np.cumsum(deg)]).astype(np.int64)

    kv_idx = np.zeros((NCORES, 16, 8 * R_tot), dtype=np.int16)
    mask = np.full((NCORES, 128, R_tot), -60000.0, dtype=np.float16)
    for c in range(NCORES):
        for g in range(G):
            ra, rb, b0 = int(RA[g]), int(RB[g]), int(base[g])
            for p in range(128):
                lane = g * 128 + p
                node = pi[c * PN + lane]
                if node < 0:
                    continue
                d_n = int(deg[node])
                if d_n == 0:
                    continue
                positions = pos_of[src_s[starts[node]: starts[node] + d_n]]
                pa = positions[positions < SPLIT]
                pb = positions[positions >= SPLIT] - SPLIT
                # A block: slots i = r*128+p at [i%16, 8*b0 + i//16]
                for r, v in enumerate(pa):
                    i = r * 128 + p
                    kv_idx[c, i % 16, 8 * b0 + i // 16] = v
                for r, v in enumerate(pb):
                    i = r * 128 + p
                    kv_idx[c, i % 16, 8 * (b0 + ra) + i // 16] = v
                mask[c, p, b0: b0 + len(pa)] = 0.0
                mask[c, p, b0 + ra: b0 + ra + len(pb)] = 0.0
    kv_idx_full = np.tile(kv_idx, (1, 8, 1))   # replicate for 8 gpsimd cores
    return pi, RA, RB, R_tot, kv_idx_full, mask, batches


def _build_program(RA, RB, R_tot, batches, L_EFF=L):
    import concourse.bass as bass
    import concourse.mybir as mybir
    from concourse import bacc, masks
    from concourse.tile import TileContext

    GATHER_ONLY = bool(os.environ.get("KB_GATHER_ONLY"))
    NO_EDGE = bool(os.environ.get("KB_NO_EDGE"))
    # 4 SWDGE queues + multi-packet: ~1.9x faster edge gathers (measured;
    # more in-flight descriptors hide HBM random-read latency)
    NQ = int(os.environ.get("KB_SWDGE_QUEUES", "4"))
    SINGLE_PACKET = os.environ.get("KB_SINGLE_PACKET", "0") != "0"

    fp16 = mybir.dt.float16
    fp32 = mybir.dt.float32
    i16 = mybir.dt.int16
    AX = mybir.AxisListType
    ALU = mybir.AluOpType
    ACTF = mybir.ActivationFunctionType

    nc = bacc.Bacc("TRN2", target_bir_lowering=False, debug=False,
                   num_devices=NCORES, num_swdge_queues=NQ)

    xownT_d = nc.dram_tensor("xownT", [D, PN], fp16, kind="ExternalInput")
    wkv_d = nc.dram_tensor("wkv", [D, L * 2 * D], fp16, kind="ExternalInput")
    wqs_d = nc.dram_tensor("wqs", [D, L * 2 * D], fp16, kind="ExternalInput")
    wout_d = nc.dram_tensor("wout", [D, OUT], fp16, kind="ExternalInput")
    kvidx_d = nc.dram_tensor("kvidx", [128, 8 * R_tot], i16, kind="ExternalInput")
    mask_d = nc.dram_tensor("mask", [128, R_tot], fp16, kind="ExternalInput")
    y_d = nc.dram_tensor("y", [PN, OUT], fp32, kind="ExternalOutput")

    kvown = [nc.dram_tensor(f"kvown{l}", [PN, 2 * D], fp16, kind="Internal")
             for l in range(L_EFF)]
    kvtab = [nc.dram_tensor(f"kvtab{l}", [NPAD, 2 * D], fp16, kind="Internal",
                            addr_space="Shared") for l in range(L_EFF)]

    base = np.concatenate([[0], np.cumsum(RA + RB)]).astype(np.int64)

    with TileContext(nc) as tc:
        with (
            tc.tile_pool(name="persist", bufs=1) as pp,
            tc.tile_pool(name="rows", bufs=3) as tp,
            tc.tile_pool(name="edge", bufs=2) as ep,
            tc.tile_pool(name="small", bufs=2) as sp,
            tc.tile_pool(name="psum", bufs=4, space="PSUM") as psp,
            tc.tile_pool(name="psumT", bufs=2, space="PSUM") as pspT,
        ):
            wkv_s = pp.tile([128, L * 2 * D], fp16, tag="wkv")
            nc.sync.dma_start(wkv_s[:], wkv_d[:])
            wqs_s = pp.tile([128, L * 2 * D], fp16, tag="wqs")
            nc.sync.dma_start(wqs_s[:], wqs_d[:])
            wout_s = pp.tile([128, OUT], fp16, tag="wout")
            nc.sync.dma_start(wout_s[:], wout_d[:])
            kvidx_s = pp.tile([128, 8 * R_tot], i16, tag="kvidx")
            nc.sync.dma_start(kvidx_s[:], kvidx_d[:])
            mask_s = pp.tile([128, R_tot], fp16, tag="mask")
            nc.sync.dma_start(mask_s[:], mask_d[:])
            ident = pp.tile([128, 128], fp16, tag="ident")
            masks.make_identity(nc, ident[:])

            qrr = [0]
            xTo = [pp.tile([128, PN], fp16, tag=f"xTo{i}", name=f"xTo{i}")
                   for i in range(2)]
            qs_all = [pp.tile([128, G * 2 * D], fp16, tag=f"qs{i}", name=f"qs{i}")
                      for i in range(2)]
            nc.sync.dma_start(xTo[0][:], xownT_d[:])

            for l in range(L_EFF):
                xT = xTo[l % 2]
                qs = qs_all[l % 2]
                # ---- own-shard k|v rows (feed the AllGather) ----
                for g in range(G):
                    ps = psp.tile([128, 2 * D], fp32, tag="ps")
                    nc.tensor.matmul(
                        ps[:], xT[:, g * 128:(g + 1) * 128],
                        wkv_s[:, l * 2 * D:(l + 1) * 2 * D])
                    row = tp.tile([128, 2 * D], fp16, tag="row")
                    nc.vector.tensor_copy(row[:], ps[:])
                    nc.sync.dma_start(kvown[l][g * 128:(g + 1) * 128, :], row[:])
                # ---- q|skip for own nodes (stays in SBUF) ----
                for g in range(G):
                    ps = psp.tile([128, 2 * D], fp32, tag="ps")
                    nc.tensor.matmul(
                        ps[:], xT[:, g * 128:(g + 1) * 128],
                        wqs_s[:, l * 2 * D:(l + 1) * 2 * D])
                    nc.vector.tensor_copy(
                        qs[:, g * 2 * D:(g + 1) * 2 * D], ps[:])

                if not os.environ.get("KB_NO_COLL"):
                    nc.gpsimd.collective_compute(
                        "AllGather", ALU.bypass,
                        replica_groups=[list(range(NCORES))],
                        ins=[kvown[l][:]], outs=[kvtab[l][:]])
                tc.strict_bb_all_engine_barrier()

                # ---- edge phase: batches of K same-R groups ----
                xTn = xTo[(l + 1) % 2]
                for (g0, K, R) in batches:
                    KR = K * R
                    b0 = int(base[g0])
                    qsb = qs[:, g0 * 2 * D:(g0 + K) * 2 * D]
                    xn = sp.tile([128, K * D], fp16, tag="xn")
                    if R == 0 or NO_EDGE or GATHER_ONLY:
                        nc.vector.tensor_scalar_max(
                            xn[:].rearrange("p (k d) -> p k d", k=K),
                            qsb.rearrange("p (k t) -> p k t", k=K)
                            [:, :, D:2 * D], 0.0)
                    if R > 0 and not NO_EDGE:
                        kv = ep.tile([128, KR, 2 * D], fp16, tag="kv")
                        # SWDGE ring holds 1024 descriptors; one instruction
                        # must stay below that (7 rounds = 896)
                        CH = int(os.environ.get("KB_CH", "7"))
                        for k in range(K):
                            ra, rb = int(RA[g0 + k]), int(RB[g0 + k])
                            bk = int(base[g0 + k])
                            for r0 in range(0, ra, CH):
                                rc = min(CH, ra - r0)
                                nc.gpsimd.dma_gather(
                                    kv[:, k * R + r0:k * R + r0 + rc, :],
                                    kvtab[l][:],
                                    kvidx_s[:, 8 * (bk + r0):
                                            8 * (bk + r0 + rc)],
                                    num_idxs=128 * rc, num_idxs_reg=128 * rc,
                                    elem_size=2 * D,
                                    queue_num=qrr[0] % NQ,
                                    single_packet=SINGLE_PACKET)
                                qrr[0] += 1
                            for r0 in range(0, rb, CH):
                                rc = min(CH, rb - r0)
                                nc.gpsimd.dma_gather(
                                    kv[:, k * R + ra + r0:
                                       k * R + ra + r0 + rc, :],
                                    kvtab[l][SPLIT:NPAD, :],
                                    kvidx_s[:, 8 * (bk + ra + r0):
                                            8 * (bk + ra + r0 + rc)],
                                    num_idxs=128 * rc, num_idxs_reg=128 * rc,
                                    elem_size=2 * D,
                                    queue_num=qrr[0] % NQ,
                                    single_packet=SINGLE_PACKET)
                                qrr[0] += 1
                    if R > 0 and not NO_EDGE and not GATHER_ONLY:
                        qk = ep.tile([128, KR, D], fp16, tag="qkmsg")
                        nc.vector.tensor_mul(
                            qk[:].rearrange("p (k r) d -> p k r d", k=K),
                            kv[:, :, 0:D].rearrange(
                                "p (k r) d -> p k r d", k=K),
                            qsb.rearrange("p (k t) -> p k t", k=K)[:, :, 0:D]
                            .unsqueeze(2).broadcast_to([128, K, R, D]))
                        qk4 = qk[:].rearrange("p kr (h c) -> p kr h c", h=H)
                        w = C
                        while w > 2:
                            hw = w // 2
                            nc.vector.tensor_tensor(
                                qk4[:, :, :, 0:hw], qk4[:, :, :, 0:hw],
                                qk4[:, :, :, hw:w], op=ALU.add)
                            w = hw
                        scm = sp.tile([128, KR * H], fp16, tag="scm")
                        nc.vector.tensor_tensor(
                            scm[:].rearrange("p (kr h) -> p kr h", h=H),
                            qk4[:, :, :, 0:1].rearrange(
                                "p kr h c -> p kr (h c)"),
                            qk4[:, :, :, 1:2].rearrange(
                                "p kr h c -> p kr (h c)"),
                            op=ALU.add)
                        nc.vector.tensor_tensor(
                            scm[:].rearrange("p (kr h) -> p kr h", h=H),
                            scm[:].rearrange("p (kr h) -> p kr h", h=H),
                            mask_s[:, b0:b0 + KR].unsqueeze(2)
                            .broadcast_to([128, KR, H]),
                            op=ALU.add)
                        mx = sp.tile([128, K * H], fp16, tag="mx")
                        nc.vector.reduce_max(
                            mx[:].rearrange("p (k h) -> p k h", k=K),
                            scm[:].rearrange("p (k r h) -> p k h r", k=K, h=H),
                            axis=AX.X)
                        pexp = sp.tile([128, KR * H], fp16, tag="pexp")
                        nc.vector.tensor_tensor(
                            pexp[:].rearrange("p (k r h) -> p k r h",
                                              k=K, h=H),
                            scm[:].rearrange("p (k r h) -> p k r h",
                                             k=K, h=H),
                            mx[:].rearrange("p (k h) -> p k h", k=K)
                            .unsqueeze(2).broadcast_to([128, K, R, H]),
                            op=ALU.subtract)
                        pexps = sp.tile([128, KR * H], fp16, tag="pexps")
                        nc.scalar.activation(pexps[:], pexp[:], ACTF.Exp)
                        pe128 = ep.tile([128, KR, D], fp16, tag="qkmsg")
                        nc.scalar.activation(
                            pe128[:].rearrange("p kr (h c) -> p kr h c", h=H),
                            pexp[:].rearrange("p (kr h) -> p kr h", h=H)
                            .unsqueeze(3).broadcast_to([128, KR, H, C]),
                            ACTF.Exp)
                        z = sp.tile([128, K * H], fp32, tag="z")
                        nc.vector.reduce_sum(
                            z[:].rearrange("p (k h) -> p k h", k=K),
                            pexps[:].rearrange("p (k r h) -> p k h r",
                                               k=K, h=H),
                            axis=AX.X)
                        zi = sp.tile([128, K * H], fp16, tag="zi")
                        with nc.allow_low_precision("alpha normalizer fp16"):
                            nc.vector.reciprocal(zi[:], z[:])
                        nc.vector.tensor_tensor(
                            pe128[:], kv[:, :, D:2 * D], pe128[:],
                            op=ALU.mult)
                        pe4 = pe128[:].rearrange("p (k r) d -> p k r d", k=K)
                        n = R
                        while n > 1:
                            hw = n // 2
                            nc.vector.tensor_tensor(
                                pe4[:, :, 0:hw, :], pe4[:, :, 0:hw, :],
                                pe4[:, :, hw:2 * hw, :], op=ALU.add)
                            if n % 2 == 1:
                                nc.vector.tensor_tensor(
                                    pe4[:, :, 0:1, :], pe4[:, :, 0:1, :],
                                    pe4[:, :, 2 * hw:2 * hw + 1, :],
                                    op=ALU.add)
                            n = hw
                        xs = sp.tile([128, K * D], fp16, tag="xs")
                        nc.vector.tensor_tensor(
                            xs[:].rearrange("p (k h c) -> p k h c",
                                            k=K, h=H),
                            pe4[:, :, 0:1, :].rearrange(
                                "p k r (h c) -> p k (r h) c", h=H),
                            zi[:].rearrange("p (k h) -> p k h", k=K)
                            .unsqueeze(3).broadcast_to([128, K, H, C]),
                            op=ALU.mult)
                        nc.vector.tensor_tensor(
                            xs[:].rearrange("p (k d) -> p k d", k=K),
                            xs[:].rearrange("p (k d) -> p k d", k=K),
                            qsb.rearrange("p (k t) -> p k t", k=K)
                            [:, :, D:2 * D],
                            op=ALU.add)
                        nc.vector.tensor_scalar_max(xn[:], xs[:], 0.0)
                    psT = pspT.tile([128, K * 128], fp16, tag="psT")
                    for k in range(K):
                        nc.tensor.matmul(
                            psT[:, k * 128:(k + 1) * 128],
                            xn[:, k * D:(k + 1) * D], ident[:],
                            is_transpose=True)
                    nc.vector.tensor_copy(
                        xTn[:, g0 * 128:(g0 + K) * 128], psT[:])

            # ---- final projection (own nodes) ----
            xT = xTo[L_EFF % 2]
            for g in range(G):
                ps = psp.tile([128, 2 * D], fp32, tag="ps")
                nc.tensor.matmul(ps[:, 0:OUT], xT[:, g * 128:(g + 1) * 128],
                                 wout_s[:])
                yo = tp.tile([128, OUT], fp32, tag="yo")
                nc.vector.tensor_copy(yo[:], ps[:, 0:OUT])
                nc.sync.dma_start(y_d[g * 128:(g + 1) * 128, :], yo[:])

    nc.compile()
    return nc


def _make_runner(nc, n_cores=NCORES):
    import jax
    from jax.sharding import Mesh, PartitionSpec
    from jax.experimental.shard_map import shard_map
    import concourse.mybir as mybir
    from concourse import bass2jax

    bass2jax.install_neuronx_cc_hook()
    partition_name = nc.partition_id_tensor.name if nc.partition_id_tensor else None
    in_names, out_names, out_avals, zero_outs = [], [], [], []
    for alloc in nc.m.functions[0].allocations:
        if not isinstance(alloc, mybir.MemoryLocationSet):
            continue
        name = alloc.memorylocations[0].name
        if alloc.kind == "ExternalInput":
            if name != partition_name:
                in_names.append(name)
        elif alloc.kind == "ExternalOutput":
            shape = tuple(alloc.tensor_shape)
            dtype = mybir.dt.np(alloc.dtype)
            out_names.append(name)
            out_avals.append(jax.core.ShapedArray(shape, dtype))
            zero_outs.append(np.zeros(shape, dtype))
    n_params = len(in_names)
    n_outs = len(out_avals)
    all_in_names = in_names + out_names + ([partition_name] if partition_name else [])
    donate = tuple(range(n_params, n_params + n_outs))

    def _body(*args):
        operands = list(args)
        if partition_name is not None:
            operands.append(bass2jax.partition_id_tensor())
        outs = bass2jax._bass_exec_p.bind(
            *operands, out_avals=tuple(out_avals), in_names=tuple(all_in_names),
            out_names=tuple(out_names), lowering_input_output_aliases=(),
            sim_require_finite=True, sim_require_nnan=True, nc=nc)
        return tuple(outs)

    devices = jax.devices()[:n_cores]
    mesh = Mesh(np.asarray(devices), ("core",))
    in_specs = (PartitionSpec("core"),) * (n_params + n_outs)
    out_specs = (PartitionSpec("core"),) * n_outs
    fn = jax.jit(shard_map(_body, mesh=mesh, in_specs=in_specs,
                           out_specs=out_specs, check_rep=False),
                 keep_unused=True)

    def run(in_maps, time_reps=0):
        concat_in = [
            np.concatenate([np.asarray(in_maps[c][nm]) for c in range(n_cores)], axis=0)
            for nm in in_names]
        concat_zeros = [np.zeros((n_cores * z.shape[0], *z.shape[1:]), z.dtype)
                        for z in zero_outs]
        args = [jax.device_put(a) for a in concat_in + concat_zeros]
        out = fn(*args)
        jax.block_until_ready(out)
        tmin = None
        if time_reps:
            ts = []
            for _ in range(time_reps):
                t0 = time.perf_counter()
                out = fn(*args)
                jax.block_until_ready(out)
                ts.append(time.perf_counter() - t0)
            tmin = min(ts)
        results = [
            {nm: np.asarray(out[i]).reshape(n_cores, *out_avals[i].shape)[c]
             for i, nm in enumerate(out_names)}
            for c in range(n_cores)]
        return results, tmin
    return run


def kernel(**inputs):
    x = np.asarray(inputs["x"], dtype=np.float32)
    edge_index = np.asarray(inputs["edge_index"])
    Wq = np.asarray(inputs["Wq"], dtype=np.float32)
    Wk = np.asarray(inputs["Wk"], dtype=np.float32)
    Wv = np.asarray(inputs["Wv"], dtype=np.float32)
    Wskip = np.asarray(inputs["Wskip"], dtype=np.float32)
    Wout = np.asarray(inputs["Wout"], dtype=np.float32)
    bout = np.asarray(inputs["bout"], dtype=np.float32)
    for b in ("bq", "bk", "bv", "bskip"):
        assert np.all(np.asarray(inputs[b]) == 0.0), f"{b} must be zero"

    if "prog" not in _cache:
        pi, RA, RB, R_tot, kv_idx, mask, batches = _host_prep(edge_index)
        nc = _build_program(RA, RB, R_tot, batches,
                            L_EFF=int(os.environ.get("KB_LAYERS", str(L))))
        run = _make_runner(nc)
        _cache["prog"] = (pi, R_tot, kv_idx, mask, run)
    pi, R_tot, kv_idx, mask, run = _cache["prog"]

    # q gets the 1/sqrt(C)=0.25 attention scale folded in
    wkv = np.transpose(np.concatenate([Wk, Wv], axis=2), (1, 0, 2)).reshape(
        D, L * 2 * D).astype(np.float16)
    wqs = np.transpose(np.concatenate([Wq * 0.25, Wskip], axis=2),
                       (1, 0, 2)).reshape(D, L * 2 * D).astype(np.float16)
    x0 = np.zeros((NPAD, D), dtype=np.float16)
    valid = pi >= 0
    x0[valid] = x[pi[valid]].astype(np.float16)
    in_maps = []
    for c in range(NCORES):
        in_maps.append({
            "xownT": np.ascontiguousarray(x0[c * PN:(c + 1) * PN].T),
            "wkv": wkv,
            "wqs": wqs,
            "wout": Wout.astype(np.float16),
            "kvidx": kv_idx[c],
            "mask": mask[c],
        })
    results, tmin = run(in_maps, time_reps=int(os.environ.get("KB_TIME_REPS", "0")))
    globals()['_last_results'] = results
    globals()['_last_run'] = run
    globals()['_last_in_maps'] = in_maps
    if tmin is not None:
        globals()['_last_wall_us'] = tmin * 1e6
        print(f"kernel exec wall (min): {tmin*1e6:.0f} us")

    out = np.zeros((N, OUT), dtype=np.float32)
    for c in range(NCORES):
        yc = results[c]["y"]
        ids = pi[c * PN:(c + 1) * PN]
        m = ids >= 0
        out[ids[m]] = yc[m]
    return out + bout[None, :]



# revision 17
# speedup vs baseline: 9.0898x; 1.9469x over previous
"""Trainium2 Bass kernel for EnhancedGNNTransformerEncoder (GNN message passing).

Sharding: dst-node sharding across 8 NeuronCores. Per-core node permutation
(degree- and A-count-sorted, padded to 6272 rows/core) makes every node-table
access affine. Edge layout: dst nodes on partitions, in-edges as near-uniform
rounds, so segment softmax/aggregation are strided DVE ops.

v2 data flow: instead of AllGathering the k|v table (25.7MB/layer), each layer
AllGathers the much smaller node features x (12.8MB, and only for layers 1-3;
layer 0's x is shipped as a replicated input), then every core redundantly
computes the FULL k|v table locally (DMA-transpose x chunks + PE matmuls, PE
is otherwise idle) and writes it to local DRAM. Per-edge k|v rows are fetched
with batched dma_gather over 4 SWDGE queues with multi-packet draining (~2x
descriptor throughput vs one queue; the gathers are HBM-latency-bound).
"""

import os
import sys
import time

sys.path.insert(0, "/opt/trn_rl_repo")

import numpy as np

N = 50000
E = 800000
D = 128
H = 8
C = 16
L = 4
OUT = 128
NCORES = 8
PERCORE = N // NCORES       # 6250
PN = 6272                   # padded per-core rows (multiple of 128)
NPAD = PN * NCORES          # 50176
G = PN // 128               # 49 groups per core
SPLIT = 5 * PN              # 31360: k|v table rows < SPLIT use gather A

_cache = {}


def _host_prep(edge_index):
    src = np.asarray(edge_index[0]).astype(np.int64)
    dst = np.asarray(edge_index[1]).astype(np.int64)
    deg = np.bincount(dst, minlength=N)
    # A-edge iff src's core <= 4 iff src < 5*PERCORE (position block < SPLIT)
    a_cnt = np.bincount(dst[src < 5 * PERCORE], minlength=N)

    pi = np.full(NPAD, -1, dtype=np.int64)
    pos_of = np.zeros(N, dtype=np.int64)
    for c in range(NCORES):
        own = np.arange(c * PERCORE, (c + 1) * PERCORE)
        # sort by the binding normalized dimension max(A/5, B/3) desc (A
        # covers 5/8 of sources, B 3/8), then by A: keeps per-group max(A)
        # and max(B) jointly tight and aligns profiles across cores (round
        # counts are global maxima over all 8 cores)
        b_cnt = deg[own] - a_cnt[own]
        binding = np.maximum(a_cnt[own] / 5.0, b_cnt / 3.0)
        order = own[np.lexsort((-(a_cnt[own] / 5.0 - b_cnt / 3.0), -binding))]
        pi[c * PN: c * PN + PERCORE] = order
        pos_of[order] = c * PN + np.arange(PERCORE)

    # per-(core, group, lane) A/B counts -> global RA/RB per group
    lane_a = np.zeros((NCORES, PN), dtype=np.int64)
    lane_b = np.zeros((NCORES, PN), dtype=np.int64)
    for c in range(NCORES):
        nodes = pi[c * PN: c * PN + PERCORE]
        lane_a[c, :PERCORE] = a_cnt[nodes]
        lane_b[c, :PERCORE] = deg[nodes] - a_cnt[nodes]
    RA = np.zeros(G, dtype=np.int64)
    RB = np.zeros(G, dtype=np.int64)
    for g in range(G):
        RA[g] = lane_a[:, g * 128:(g + 1) * 128].max()
        RB[g] = lane_b[:, g * 128:(g + 1) * 128].max()
    RB += (RA + RB) % 2          # even round count per group
    Rg = RA + RB

    # reorder the 49 groups (blocks of 128 lanes) so equal-R groups are
    # adjacent: the kernel then processes several same-R groups in one set
    # of wide DVE ops. A/B membership only depends on the source core, so
    # permuting whole blocks within a core is free.
    perm = np.argsort(-Rg, kind="stable")
    RA, RB, Rg = RA[perm], RB[perm], Rg[perm]
    pi2 = np.full(NPAD, -1, dtype=np.int64)
    for c in range(NCORES):
        blocks = pi[c * PN:(c + 1) * PN].reshape(G, 128)
        pi2[c * PN:(c + 1) * PN] = blocks[perm].reshape(-1)
    pi = pi2
    for c in range(NCORES):
        own_mask = pi[c * PN:(c + 1) * PN] >= 0
        pos_of[pi[c * PN:(c + 1) * PN][own_mask]] = \
            c * PN + np.nonzero(own_mask)[0]

    # batches of consecutive equal-R groups, capped at RCAP total rounds
    RCAP = 48
    batches = []
    g = 0
    while g < G:
        R = int(Rg[g])
        k = 1
        while (g + k < G and int(Rg[g + k]) == R and R > 0
               and (k + 1) * R <= RCAP):
            k += 1
        batches.append((g, k, R))
        g += k

    base = np.concatenate([[0], np.cumsum(Rg)]).astype(np.int64)
    R_tot = int(base[-1])

    order_e = np.lexsort((src, dst))
    src_s = src[order_e]
    starts = np.concatenate([[0], np.cumsum(deg)]).astype(np.int64)

    kv_idx = np.zeros((NCORES, 16, 8 * R_tot), dtype=np.int16)
    mask = np.full((NCORES, 128, R_tot), -60000.0, dtype=np.float16)
    for c in range(NCORES):
        for g in range(G):
            ra, rb, b0 = int(RA[g]), int(RB[g]), int(base[g])
            for p in range(128):
                lane = g * 128 + p
                node = pi[c * PN + lane]
                if node < 0:
                    continue
                d_n = int(deg[node])
                if d_n == 0:
                    continue
                positions = pos_of[src_s[starts[node]: starts[node] + d_n]]
                pa = positions[positions < SPLIT]
                pb = positions[positions >= SPLIT] - SPLIT
                # A block: slots i = r*128+p at [i%16, 8*b0 + i//16]
                for r, v in enumerate(pa):
                    i = r * 128 + p
                    kv_idx[c, i % 16, 8 * b0 + i // 16] = v
                for r, v in enumerate(pb):
                    i = r * 128 + p
                    kv_idx[c, i % 16, 8 * (b0 + ra) + i // 16] = v
                mask[c, p, b0: b0 + len(pa)] = 0.0
                mask[c, p, b0 + ra: b0 + ra + len(pb)] = 0.0
    kv_idx_full = np.tile(kv_idx, (1, 8, 1))   # replicate for 8 gpsimd cores
    return pi, RA, RB, R_tot, kv_idx_full, mask, batches


def _build_program(RA, RB, R_tot, batches, L_EFF=L):
    import concourse.bass as bass
    import concourse.mybir as mybir
    from concourse import bacc, masks
    from concourse.tile import TileContext

    GATHER_ONLY = bool(os.environ.get("KB_GATHER_ONLY"))
    NO_EDGE = bool(os.environ.get("KB_NO_EDGE"))
    NO_COLL = bool(os.environ.get("KB_NO_COLL"))
    NO_KVBUILD = bool(os.environ.get("KB_NO_KVBUILD"))
    # 4 SWDGE queues + multi-packet: ~1.9x faster edge gathers (measured;
    # more in-flight descriptors hide HBM random-read latency)
    NQ = int(os.environ.get("KB_SWDGE_QUEUES", "4"))
    SINGLE_PACKET = os.environ.get("KB_SINGLE_PACKET", "0") != "0"
    FUSE_EXP = os.environ.get("KB_FUSE_EXP", "1") != "0"
    DOT_REDUCE = os.environ.get("KB_DOT_REDUCE", "1") != "0"

    fp16 = mybir.dt.float16
    fp32 = mybir.dt.float32
    i16 = mybir.dt.int16
    AX = mybir.AxisListType
    ALU = mybir.AluOpType
    ACTF = mybir.ActivationFunctionType

    nc = bacc.Bacc("TRN2", target_bir_lowering=False, debug=False,
                   num_devices=NCORES, num_swdge_queues=NQ)

    xownT_d = nc.dram_tensor("xownT", [D, PN], fp16, kind="ExternalInput")
    xfull0_d = nc.dram_tensor("xfull0", [NPAD, D], fp16, kind="ExternalInput")
    wkv_d = nc.dram_tensor("wkv", [D, L * 2 * D], fp16, kind="ExternalInput")
    wqs_d = nc.dram_tensor("wqs", [D, L * 2 * D], fp16, kind="ExternalInput")
    wout_d = nc.dram_tensor("wout", [D, OUT], fp16, kind="ExternalInput")
    kvidx_d = nc.dram_tensor("kvidx", [128, 8 * R_tot], i16, kind="ExternalInput")
    mask_d = nc.dram_tensor("mask", [128, R_tot], fp16, kind="ExternalInput")
    y_d = nc.dram_tensor("y", [PN, OUT], fp32, kind="ExternalOutput")

    kvtab = [nc.dram_tensor(f"kvtab{l}", [NPAD, 2 * D], fp16, kind="Internal")
             for l in range(L_EFF)]
    # node-major own x produced by edge phase l-1, and its AllGather
    xnm = [None] + [nc.dram_tensor(f"xnm{l}", [PN, D], fp16, kind="Internal")
                    for l in range(1, L_EFF)]
    xfull = [xfull0_d] + [
        nc.dram_tensor(f"xfull{l}", [NPAD, D], fp16, kind="Internal",
                       addr_space="Shared") for l in range(1, L_EFF)]

    base = np.concatenate([[0], np.cumsum(RA + RB)]).astype(np.int64)

    with TileContext(nc) as tc:
        with (
            tc.tile_pool(name="persist", bufs=1) as pp,
            tc.tile_pool(name="xchunk", bufs=2) as xp,
            tc.tile_pool(name="rows", bufs=4) as tp,
            tc.tile_pool(name="edge", bufs=2) as ep,
            tc.tile_pool(name="small", bufs=2) as sp,
            tc.tile_pool(name="psum", bufs=4, space="PSUM") as psp,
            tc.tile_pool(name="psumT", bufs=2, space="PSUM") as pspT,
        ):
            wkv_s = pp.tile([128, L * 2 * D], fp16, tag="wkv")
            nc.sync.dma_start(wkv_s[:], wkv_d[:])
            wqs_s = pp.tile([128, L * 2 * D], fp16, tag="wqs")
            nc.sync.dma_start(wqs_s[:], wqs_d[:])
            wout_s = pp.tile([128, OUT], fp16, tag="wout")
            nc.sync.dma_start(wout_s[:], wout_d[:])
            kvidx_s = pp.tile([128, 8 * R_tot], i16, tag="kvidx")
            nc.sync.dma_start(kvidx_s[:], kvidx_d[:])
            mask_s = pp.tile([128, R_tot], fp16, tag="mask")
            nc.sync.dma_start(mask_s[:], mask_d[:])
            ident = pp.tile([128, 128], fp16, tag="ident")
            masks.make_identity(nc, ident[:])

            qrr = [0]
            xTo = [pp.tile([128, PN], fp16, tag=f"xTo{i}", name=f"xTo{i}")
                   for i in range(2)]
            qs = pp.tile([128, G * 2 * D], fp16, tag="qs", name="qs")
            nc.sync.dma_start(xTo[0][:], xownT_d[:])

            for l in range(L_EFF):
                xT = xTo[l % 2]
                # ---- full k|v table, computed locally from replicated x ----
                if not NO_KVBUILD:
                    GH = 7  # groups per staged kvtab DMA (49 = 7x7)
                    for b in range(NCORES):
                        xfb = xp.tile([128, PN], fp16, tag="xfb")
                        nc.sync.dma_start_transpose(
                            xfb[:], xfull[l][b * PN:(b + 1) * PN, :])
                        for gh in range(G // GH):
                            stage = tp.tile([128, GH * 2 * D], fp16,
                                            tag="kvstage")
                            for g2 in range(GH):
                                g = gh * GH + g2
                                ps = psp.tile([128, 2 * D], fp32, tag="ps")
                                nc.tensor.matmul(
                                    ps[:], xfb[:, g * 128:(g + 1) * 128],
                                    wkv_s[:, l * 2 * D:(l + 1) * 2 * D])
                                dst = stage[:, g2 * 2 * D:(g2 + 1) * 2 * D]
                                if g2 % 2 == 0:
                                    nc.vector.tensor_copy(dst, ps[:])
                                else:
                                    nc.scalar.copy(dst, ps[:])
                            r0 = (b * G + gh * GH) * 128
                            nc.sync.dma_start(
                                kvtab[l][r0:r0 + GH * 128, :]
                                .rearrange("(g p) d -> p g d", p=128),
                                stage[:].rearrange("p (g d) -> p g d", g=GH))
                # ---- q|skip for own nodes (stays in SBUF) ----
                for g in range(G):
                    ps = psp.tile([128, 2 * D], fp32, tag="ps")
                    nc.tensor.matmul(
                        ps[:], xT[:, g * 128:(g + 1) * 128],
                        wqs_s[:, l * 2 * D:(l + 1) * 2 * D])
                    nc.vector.tensor_copy(
                        qs[:, g * 2 * D:(g + 1) * 2 * D], ps[:])

                # ---- edge phase: batches of K same-R groups ----
                xTn = xTo[(l + 1) % 2]
                for (g0, K, R) in batches:
                    KR = K * R
                    b0 = int(base[g0])
                    qsb = qs[:, g0 * 2 * D:(g0 + K) * 2 * D]
                    xn = sp.tile([128, K * D], fp16, tag="xn")
                    if R == 0 or NO_EDGE or GATHER_ONLY:
                        nc.vector.tensor_scalar_max(
                            xn[:].rearrange("p (k d) -> p k d", k=K),
                            qsb.rearrange("p (k t) -> p k t", k=K)
                            [:, :, D:2 * D], 0.0)
                    if R > 0 and not NO_EDGE:
                        kv = ep.tile([128, KR, 2 * D], fp16, tag="kv")
                        # SWDGE ring holds 1024 descriptors; one instruction
                        # must stay below that (7 rounds = 896)
                        CH = int(os.environ.get("KB_CH", "7"))
                        for k in range(K):
                            ra, rb = int(RA[g0 + k]), int(RB[g0 + k])
                            bk = int(base[g0 + k])
                            for r0 in range(0, ra, CH):
                                rc = min(CH, ra - r0)
                                nc.gpsimd.dma_gather(
                                    kv[:, k * R + r0:k * R + r0 + rc, :],
                                    kvtab[l][:],
                                    kvidx_s[:, 8 * (bk + r0):
                                            8 * (bk + r0 + rc)],
                                    num_idxs=128 * rc, num_idxs_reg=128 * rc,
                                    elem_size=2 * D,
                                    queue_num=qrr[0] % NQ,
                                    single_packet=SINGLE_PACKET)
                                qrr[0] += 1
                            for r0 in range(0, rb, CH):
                                rc = min(CH, rb - r0)
                                nc.gpsimd.dma_gather(
                                    kv[:, k * R + ra + r0:
                                       k * R + ra + r0 + rc, :],
                                    kvtab[l][SPLIT:NPAD, :],
                                    kvidx_s[:, 8 * (bk + ra + r0):
                                            8 * (bk + ra + r0 + rc)],
                                    num_idxs=128 * rc, num_idxs_reg=128 * rc,
                                    elem_size=2 * D,
                                    queue_num=qrr[0] % NQ,
                                    single_packet=SINGLE_PACKET)
                                qrr[0] += 1
                    if R > 0 and not NO_EDGE and not GATHER_ONLY:
                        qk = ep.tile([128, KR, D], fp16, tag="qkmsg")
                        nc.vector.tensor_mul(
                            qk[:].rearrange("p (k r) d -> p k r d", k=K),
                            kv[:, :, 0:D].rearrange(
                                "p (k r) d -> p k r d", k=K),
                            qsb.rearrange("p (k t) -> p k t", k=K)[:, :, 0:D]
                            .unsqueeze(2).broadcast_to([128, K, R, D]))
                        scm = sp.tile([128, KR * H], fp16, tag="scm")
                        if DOT_REDUCE:
                            # per-(edge,head) dot: single contiguous-X reduce
                            # (fp16 out matches the fp16 tree-add it replaces)
                            with nc.allow_low_precision("edge dot fp16"):
                                nc.vector.reduce_sum(
                                    scm[:].rearrange("p (kr h) -> p kr h",
                                                     h=H),
                                    qk[:].rearrange("p kr (h c) -> p (kr h) c",
                                                    h=H),
                                    axis=AX.X)
                        else:
                            qk4 = qk[:].rearrange("p kr (h c) -> p kr h c",
                                                  h=H)
                            w = C
                            while w > 2:
                                hw = w // 2
                                nc.vector.tensor_tensor(
                                    qk4[:, :, :, 0:hw], qk4[:, :, :, 0:hw],
                                    qk4[:, :, :, hw:w], op=ALU.add)
                                w = hw
                            nc.vector.tensor_tensor(
                                scm[:].rearrange("p (kr h) -> p kr h", h=H),
                                qk4[:, :, :, 0:1].rearrange(
                                    "p kr h c -> p kr (h c)"),
                                qk4[:, :, :, 1:2].rearrange(
                                    "p kr h c -> p kr (h c)"),
                                op=ALU.add)
                        nc.vector.tensor_tensor(
                            scm[:].rearrange("p (kr h) -> p kr h", h=H),
                            scm[:].rearrange("p (kr h) -> p kr h", h=H),
                            mask_s[:, b0:b0 + KR].unsqueeze(2)
                            .broadcast_to([128, KR, H]),
                            op=ALU.add)
                        mx = sp.tile([128, K * H], fp16, tag="mx")
                        nc.vector.reduce_max(
                            mx[:].rearrange("p (k h) -> p k h", k=K),
                            scm[:].rearrange("p (k r h) -> p k h r", k=K, h=H),
                            axis=AX.X)
                        pexp = sp.tile([128, KR * H], fp16, tag="pexp")
                        nc.vector.tensor_tensor(
                            pexp[:].rearrange("p (k r h) -> p k r h",
                                              k=K, h=H),
                            scm[:].rearrange("p (k r h) -> p k r h",
                                             k=K, h=H),
                            mx[:].rearrange("p (k h) -> p k h", k=K)
                            .unsqueeze(2).broadcast_to([128, K, R, H]),
                            op=ALU.subtract)
                        pexps = sp.tile([128, KR * H], fp16, tag="pexps")
                        nc.scalar.activation(pexps[:], pexp[:], ACTF.Exp)
                        pe128 = ep.tile([128, KR, D], fp16, tag="qkmsg")
                        if FUSE_EXP:
                            # msg = v * exp(score-max), exp broadcast over C
                            nc.vector.tensor_tensor(
                                pe128[:].rearrange("p kr (h c) -> p kr h c",
                                                   h=H),
                                kv[:, :, D:2 * D].rearrange(
                                    "p kr (h c) -> p kr h c", h=H),
                                pexps[:].rearrange("p (kr h) -> p kr h", h=H)
                                .unsqueeze(3).broadcast_to([128, KR, H, C]),
                                op=ALU.mult)
                        else:
                            nc.scalar.activation(
                                pe128[:].rearrange("p kr (h c) -> p kr h c",
                                                   h=H),
                                pexp[:].rearrange("p (kr h) -> p kr h", h=H)
                                .unsqueeze(3).broadcast_to([128, KR, H, C]),
                                ACTF.Exp)
                            nc.vector.tensor_tensor(
                                pe128[:], kv[:, :, D:2 * D], pe128[:],
                                op=ALU.mult)
                        z = sp.tile([128, K * H], fp32, tag="z")
                        nc.vector.reduce_sum(
                            z[:].rearrange("p (k h) -> p k h", k=K),
                            pexps[:].rearrange("p (k r h) -> p k h r",
                                               k=K, h=H),
                            axis=AX.X)
                        zi = sp.tile([128, K * H], fp16, tag="zi")
                        with nc.allow_low_precision("alpha normalizer fp16"):
                            nc.vector.reciprocal(zi[:], z[:])
                        pe4 = pe128[:].rearrange("p (k r) d -> p k r d", k=K)
                        n = R
                        while n > 1:
                            hw = n // 2
                            nc.vector.tensor_tensor(
                                pe4[:, :, 0:hw, :], pe4[:, :, 0:hw, :],
                                pe4[:, :, hw:2 * hw, :], op=ALU.add)
                            if n % 2 == 1:
                                nc.vector.tensor_tensor(
                                    pe4[:, :, 0:1, :], pe4[:, :, 0:1, :],
                                    pe4[:, :, 2 * hw:2 * hw + 1, :],
                                    op=ALU.add)
                            n = hw
                        xs = sp.tile([128, K * D], fp16, tag="xs")
                        nc.vector.tensor_tensor(
                            xs[:].rearrange("p (k h c) -> p k h c",
                                            k=K, h=H),
                            pe4[:, :, 0:1, :].rearrange(
                                "p k r (h c) -> p k (r h) c", h=H),
                            zi[:].rearrange("p (k h) -> p k h", k=K)
                            .unsqueeze(3).broadcast_to([128, K, H, C]),
                            op=ALU.mult)
                        nc.vector.tensor_tensor(
                            xs[:].rearrange("p (k d) -> p k d", k=K),
                            xs[:].rearrange("p (k d) -> p k d", k=K),
                            qsb.rearrange("p (k t) -> p k t", k=K)
                            [:, :, D:2 * D],
                            op=ALU.add)
                        nc.vector.tensor_scalar_max(xn[:], xs[:], 0.0)
                    # next layer needs own x feature-major (q|skip) and, for
                    # layers with a following AllGather, node-major in DRAM
                    psT = pspT.tile([128, K * 128], fp16, tag="psT")
                    for k in range(K):
                        nc.tensor.matmul(
                            psT[:, k * 128:(k + 1) * 128],
                            xn[:, k * D:(k + 1) * D], ident[:],
                            is_transpose=True)
                    nc.vector.tensor_copy(
                        xTn[:, g0 * 128:(g0 + K) * 128], psT[:])
                    if l + 1 < L_EFF:
                        nc.sync.dma_start(
                            xnm[l + 1][g0 * 128:(g0 + K) * 128, :]
                            .rearrange("(k p) d -> p k d", p=128),
                            xn[:].rearrange("p (k d) -> p k d", k=K))

                if l + 1 < L_EFF:
                    if not NO_COLL:
                        nc.gpsimd.collective_compute(
                            "AllGather", ALU.bypass,
                            replica_groups=[list(range(NCORES))],
                            ins=[xnm[l + 1][:]], outs=[xfull[l + 1][:]])
                    tc.strict_bb_all_engine_barrier()

            # ---- final projection (own nodes) ----
            xT = xTo[L_EFF % 2]
            for g in range(G):
                ps = psp.tile([128, 2 * D], fp32, tag="ps")
                nc.tensor.matmul(ps[:, 0:OUT], xT[:, g * 128:(g + 1) * 128],
                                 wout_s[:])
                yo = tp.tile([128, OUT], fp32, tag="yo")
                nc.vector.tensor_copy(yo[:], ps[:, 0:OUT])
                nc.sync.dma_start(y_d[g * 128:(g + 1) * 128, :], yo[:])

    nc.compile()
    return nc


def _make_runner(nc, n_cores=NCORES):
    import jax
    from jax.sharding import Mesh, PartitionSpec
    from jax.experimental.shard_map import shard_map
    import concourse.mybir as mybir
    from concourse import bass2jax

    bass2jax.install_neuronx_cc_hook()
    partition_name = nc.partition_id_tensor.name if nc.partition_id_tensor else None
    in_names, out_names, out_avals, zero_outs = [], [], [], []
    for alloc in nc.m.functions[0].allocations:
        if not isinstance(alloc, mybir.MemoryLocationSet):
            continue
        name = alloc.memorylocations[0].name
        if alloc.kind == "ExternalInput":
            if name != partition_name:
                in_names.append(name)
        elif alloc.kind == "ExternalOutput":
            shape = tuple(alloc.tensor_shape)
            dtype = mybir.dt.np(alloc.dtype)
            out_names.append(name)
            out_avals.append(jax.core.ShapedArray(shape, dtype))
            zero_outs.append(np.zeros(shape, dtype))
    n_params = len(in_names)
    n_outs = len(out_avals)
    all_in_names = in_names + out_names + ([partition_name] if partition_name else [])
    donate = tuple(range(n_params, n_params + n_outs))

    def _body(*args):
        operands = list(args)
        if partition_name is not None:
            operands.append(bass2jax.partition_id_tensor())
        outs = bass2jax._bass_exec_p.bind(
            *operands, out_avals=tuple(out_avals), in_names=tuple(all_in_names),
            out_names=tuple(out_names), lowering_input_output_aliases=(),
            sim_require_finite=True, sim_require_nnan=True, nc=nc)
        return tuple(outs)

    devices = jax.devices()[:n_cores]
    mesh = Mesh(np.asarray(devices), ("core",))
    in_specs = (PartitionSpec("core"),) * (n_params + n_outs)
    out_specs = (PartitionSpec("core"),) * n_outs
    fn = jax.jit(shard_map(_body, mesh=mesh, in_specs=in_specs,
                           out_specs=out_specs, check_rep=False),
                 keep_unused=True)

    def run(in_maps, time_reps=0):
        concat_in = [
            np.concatenate([np.asarray(in_maps[c][nm]) for c in range(n_cores)], axis=0)
            for nm in in_names]
        concat_zeros = [np.zeros((n_cores * z.shape[0], *z.shape[1:]), z.dtype)
                        for z in zero_outs]
        args = [jax.device_put(a) for a in concat_in + concat_zeros]
        out = fn(*args)
        jax.block_until_ready(out)
        tmin = None
        if time_reps:
            ts = []
            for _ in range(time_reps):
                t0 = time.perf_counter()
                out = fn(*args)
                jax.block_until_ready(out)
                ts.append(time.perf_counter() - t0)
            tmin = min(ts)
        results = [
            {nm: np.asarray(out[i]).reshape(n_cores, *out_avals[i].shape)[c]
             for i, nm in enumerate(out_names)}
            for c in range(n_cores)]
        return results, tmin
    return run


def kernel(**inputs):
    x = np.asarray(inputs["x"], dtype=np.float32)
    edge_index = np.asarray(inputs["edge_index"])
    Wq = np.asarray(inputs["Wq"], dtype=np.float32)
    Wk = np.asarray(inputs["Wk"], dtype=np.float32)
    Wv = np.asarray(inputs["Wv"], dtype=np.float32)
    Wskip = np.asarray(inputs["Wskip"], dtype=np.float32)
    Wout = np.asarray(inputs["Wout"], dtype=np.float32)
    bout = np.asarray(inputs["bout"], dtype=np.float32)
    for b in ("bq", "bk", "bv", "bskip"):
        assert np.all(np.asarray(inputs[b]) == 0.0), f"{b} must be zero"

    if "prog" not in _cache:
        pi, RA, RB, R_tot, kv_idx, mask, batches = _host_prep(edge_index)
        nc = _build_program(RA, RB, R_tot, batches,
                            L_EFF=int(os.environ.get("KB_LAYERS", str(L))))
        run = _make_runner(nc)
        _cache["prog"] = (pi, R_tot, kv_idx, mask, run)
    pi, R_tot, kv_idx, mask, run = _cache["prog"]

    # q gets the 1/sqrt(C)=0.25 attention scale folded in
    wkv = np.transpose(np.concatenate([Wk, Wv], axis=2), (1, 0, 2)).reshape(
        D, L * 2 * D).astype(np.float16)
    wqs = np.transpose(np.concatenate([Wq * 0.25, Wskip], axis=2),
                       (1, 0, 2)).reshape(D, L * 2 * D).astype(np.float16)
    x0 = np.zeros((NPAD, D), dtype=np.float16)
    valid = pi >= 0
    x0[valid] = x[pi[valid]].astype(np.float16)
    in_maps = []
    for c in range(NCORES):
        in_maps.append({
            "xownT": np.ascontiguousarray(x0[c * PN:(c + 1) * PN].T),
            "xfull0": x0,
            "wkv": wkv,
            "wqs": wqs,
            "wout": Wout.astype(np.float16),
            "kvidx": kv_idx[c],
            "mask": mask[c],
        })
    results, tmin = run(in_maps, time_reps=int(os.environ.get("KB_TIME_REPS", "0")))
    globals()['_last_results'] = results
    globals()['_last_run'] = run
    globals()['_last_in_maps'] = in_maps
    if tmin is not None:
        globals()['_last_wall_us'] = tmin * 1e6
        print(f"kernel exec wall (min): {tmin*1e6:.0f} us")

    out = np.zeros((N, OUT), dtype=np.float32)
    for c in range(NCORES):
        yc = results[c]["y"]
        ids = pi[c * PN:(c + 1) * PN]
        m = ids >= 0
        out[ids[m]] = yc[m]
    return out + bout[None, :]


# revision 32
# speedup vs baseline: 13.7531x; 1.5130x over previous
"""Trainium2 Bass kernel for EnhancedGNNTransformerEncoder (GNN message passing).

Sharding: dst-node sharding across 8 NeuronCores. Per-core node permutation
(degree- and A-count-sorted, padded to 6272 rows/core) makes every node-table
access affine. Edge layout: dst nodes on partitions, in-edges as near-uniform
rounds, so segment softmax/aggregation are strided DVE ops.

v2 data flow: instead of AllGathering the k|v table (25.7MB/layer), each layer
AllGathers the much smaller node features x (12.8MB, and only for layers 1-3;
layer 0's x is shipped as a replicated input), then every core redundantly
computes the FULL k|v table locally (DMA-transpose x chunks + PE matmuls, PE
is otherwise idle) and writes it to local DRAM. Per-edge k|v rows are fetched
with batched dma_gather over 4 SWDGE queues with multi-packet draining (~2x
descriptor throughput vs one queue; the gathers are HBM-latency-bound).
"""

import os
import sys
import time

sys.path.insert(0, "/opt/trn_rl_repo")

import numpy as np

N = 50000
E = 800000
D = 128
H = 8
C = 16
L = 4
OUT = 128
NCORES = 8
PERCORE = N // NCORES       # 6250
PN = 6272                   # padded per-core rows (multiple of 128)
NPAD = PN * NCORES          # 50176
G = PN // 128               # 49 groups per core
SPLIT = 5 * PN              # 31360: k|v table rows < SPLIT use gather A

_cache = {}


def _host_prep(edge_index):
    src = np.asarray(edge_index[0]).astype(np.int64)
    dst = np.asarray(edge_index[1]).astype(np.int64)
    deg = np.bincount(dst, minlength=N)
    # A-edge iff src's core <= 4 iff src < 5*PERCORE (position block < SPLIT)
    a_cnt = np.bincount(dst[src < 5 * PERCORE], minlength=N)

    pi = np.full(NPAD, -1, dtype=np.int64)
    pos_of = np.zeros(N, dtype=np.int64)
    for c in range(NCORES):
        own = np.arange(c * PERCORE, (c + 1) * PERCORE)
        # sort by the binding normalized dimension max(A/5, B/3) desc (A
        # covers 5/8 of sources, B 3/8), then by A: keeps per-group max(A)
        # and max(B) jointly tight and aligns profiles across cores (round
        # counts are global maxima over all 8 cores)
        b_cnt = deg[own] - a_cnt[own]
        binding = np.maximum(a_cnt[own] / 5.0, b_cnt / 3.0)
        order = own[np.lexsort((-(a_cnt[own] / 5.0 - b_cnt / 3.0), -binding))]
        pi[c * PN: c * PN + PERCORE] = order
        pos_of[order] = c * PN + np.arange(PERCORE)

    # per-(core, group, lane) A/B counts -> global RA/RB per group
    lane_a = np.zeros((NCORES, PN), dtype=np.int64)
    lane_b = np.zeros((NCORES, PN), dtype=np.int64)
    for c in range(NCORES):
        nodes = pi[c * PN: c * PN + PERCORE]
        lane_a[c, :PERCORE] = a_cnt[nodes]
        lane_b[c, :PERCORE] = deg[nodes] - a_cnt[nodes]
    RA = np.zeros(G, dtype=np.int64)
    RB = np.zeros(G, dtype=np.int64)
    for g in range(G):
        RA[g] = lane_a[:, g * 128:(g + 1) * 128].max()
        RB[g] = lane_b[:, g * 128:(g + 1) * 128].max()
    RB += (RA + RB) % 2          # even round count per group
    Rg = RA + RB

    # reorder the 49 groups (blocks of 128 lanes) so equal-R groups are
    # adjacent: the kernel then processes several same-R groups in one set
    # of wide DVE ops. A/B membership only depends on the source core, so
    # permuting whole blocks within a core is free.
    perm = np.argsort(-Rg, kind="stable")
    RA, RB, Rg = RA[perm], RB[perm], Rg[perm]
    pi2 = np.full(NPAD, -1, dtype=np.int64)
    for c in range(NCORES):
        blocks = pi[c * PN:(c + 1) * PN].reshape(G, 128)
        pi2[c * PN:(c + 1) * PN] = blocks[perm].reshape(-1)
    pi = pi2
    for c in range(NCORES):
        own_mask = pi[c * PN:(c + 1) * PN] >= 0
        pos_of[pi[c * PN:(c + 1) * PN][own_mask]] = \
            c * PN + np.nonzero(own_mask)[0]

    # batches of consecutive equal-R groups, capped at RCAP total rounds
    RCAP = 48
    batches = []
    g = 0
    while g < G:
        R = int(Rg[g])
        k = 1
        while (g + k < G and int(Rg[g + k]) == R and R > 0
               and (k + 1) * R <= RCAP):
            k += 1
        batches.append((g, k, R))
        g += k

    base = np.concatenate([[0], np.cumsum(Rg)]).astype(np.int64)
    R_tot = int(base[-1])

    order_e = np.lexsort((src, dst))
    src_s = src[order_e]
    starts = np.concatenate([[0], np.cumsum(deg)]).astype(np.int64)

    kv_idx = np.zeros((NCORES, 16, 8 * R_tot), dtype=np.int16)
    mask = np.full((NCORES, 128, R_tot), -60000.0, dtype=np.float16)
    for c in range(NCORES):
        for g in range(G):
            ra, rb, b0 = int(RA[g]), int(RB[g]), int(base[g])
            for p in range(128):
                lane = g * 128 + p
                node = pi[c * PN + lane]
                if node < 0:
                    continue
                d_n = int(deg[node])
                if d_n == 0:
                    continue
                positions = pos_of[src_s[starts[node]: starts[node] + d_n]]
                pa = positions[positions < SPLIT]
                pb = positions[positions >= SPLIT] - SPLIT
                # A block: slots i = r*128+p at [i%16, 8*b0 + i//16]
                for r, v in enumerate(pa):
                    i = r * 128 + p
                    kv_idx[c, i % 16, 8 * b0 + i // 16] = v
                for r, v in enumerate(pb):
                    i = r * 128 + p
                    kv_idx[c, i % 16, 8 * (b0 + ra) + i // 16] = v
                mask[c, p, b0: b0 + len(pa)] = 0.0
                mask[c, p, b0 + ra: b0 + ra + len(pb)] = 0.0
    kv_idx_full = np.tile(kv_idx, (1, 8, 1))   # replicate for 8 gpsimd cores
    return pi, RA, RB, R_tot, kv_idx_full, mask, batches


def _build_program(RA, RB, R_tot, batches, L_EFF=L):
    import concourse.bass as bass
    import concourse.mybir as mybir
    from concourse import bacc, masks
    from concourse.tile import TileContext

    GATHER_ONLY = bool(os.environ.get("KB_GATHER_ONLY"))
    NO_EDGE = bool(os.environ.get("KB_NO_EDGE"))
    NO_COLL = bool(os.environ.get("KB_NO_COLL"))
    NO_KVBUILD = bool(os.environ.get("KB_NO_KVBUILD"))
    # 4 SWDGE queues + multi-packet: ~1.9x faster edge gathers (measured;
    # more in-flight descriptors hide HBM random-read latency)
    NQ = int(os.environ.get("KB_SWDGE_QUEUES", "4"))
    SINGLE_PACKET = os.environ.get("KB_SINGLE_PACKET", "0") != "0"
    FUSE_EXP = os.environ.get("KB_FUSE_EXP", "1") != "0"
    DOT_REDUCE = os.environ.get("KB_DOT_REDUCE", "1") != "0"
    # timing-only diagnostics (wrong numerics):
    GATHER_HALF = os.environ.get("KB_GATHER_HALF", "")
    COLL_FIRST = bool(os.environ.get("KB_COLL_FIRST"))
    if GATHER_HALF == "bytes":
        assert GATHER_ONLY, "KB_GATHER_HALF=bytes needs KB_GATHER_ONLY"

    fp16 = mybir.dt.float16
    fp32 = mybir.dt.float32
    i16 = mybir.dt.int16
    AX = mybir.AxisListType
    ALU = mybir.AluOpType
    ACTF = mybir.ActivationFunctionType

    nc = bacc.Bacc("TRN2", target_bir_lowering=False, debug=False,
                   num_devices=NCORES, num_swdge_queues=NQ)

    xownT_d = nc.dram_tensor("xownT", [D, PN], fp16, kind="ExternalInput")
    # feature-major full x0: 8 stacked [D, PN] blocks, block b = core b's slice
    xfull0_d = nc.dram_tensor("xfull0", [NCORES * D, PN], fp16,
                              kind="ExternalInput")
    wkv_d = nc.dram_tensor("wkv", [D, L * 2 * D], fp16, kind="ExternalInput")
    wqs_d = nc.dram_tensor("wqs", [D, L * 2 * D], fp16, kind="ExternalInput")
    wout_d = nc.dram_tensor("wout", [D, OUT], fp16, kind="ExternalInput")
    kvidx_d = nc.dram_tensor("kvidx", [128, 8 * R_tot], i16, kind="ExternalInput")
    mask_d = nc.dram_tensor("mask", [128, R_tot], fp16, kind="ExternalInput")
    y_d = nc.dram_tensor("y", [PN, OUT], fp32, kind="ExternalOutput")

    kvtab = [nc.dram_tensor(f"kvtab{l}", [NPAD, 2 * D], fp16, kind="Internal")
             for l in range(L_EFF)]
    # feature-major own x produced by edge phase l-1, and its AllGather
    # (concat of [D, PN] shards -> block b of the output = core b's xT)
    xnm = [None] + [nc.dram_tensor(f"xnmT{l}", [D, PN], fp16, kind="Internal")
                    for l in range(1, L_EFF)]
    xfull = [xfull0_d] + [
        nc.dram_tensor(f"xfullT{l}", [NCORES * D, PN], fp16, kind="Internal",
                       addr_space="Shared") for l in range(1, L_EFF)]

    base = np.concatenate([[0], np.cumsum(RA + RB)]).astype(np.int64)

    with TileContext(nc) as tc:
        with (
            tc.tile_pool(name="persist", bufs=1) as pp,
            tc.tile_pool(name="xchunk",
                         bufs=int(os.environ.get("KB_XC_BUFS", "2"))) as xp,
            tc.tile_pool(name="rows", bufs=3) as tp,
            tc.tile_pool(name="edgekv",
                         bufs=int(os.environ.get("KB_KV_BUFS", "2"))) as ep,
            tc.tile_pool(name="edgeqk",
                         bufs=int(os.environ.get("KB_QK_BUFS", "2"))) as eq,
            tc.tile_pool(name="small", bufs=2) as sp,
            tc.tile_pool(name="psum", bufs=4, space="PSUM") as psp,
            tc.tile_pool(name="psumT", bufs=2, space="PSUM") as pspT,
        ):
            wkv_s = pp.tile([128, L * 2 * D], fp16, tag="wkv")
            nc.sync.dma_start(wkv_s[:], wkv_d[:])
            wqs_s = pp.tile([128, L * 2 * D], fp16, tag="wqs")
            nc.sync.dma_start(wqs_s[:], wqs_d[:])
            wout_s = pp.tile([128, OUT], fp16, tag="wout")
            nc.sync.dma_start(wout_s[:], wout_d[:])
            kvidx_s = pp.tile([128, 8 * R_tot], i16, tag="kvidx")
            nc.sync.dma_start(kvidx_s[:], kvidx_d[:])
            mask_s = pp.tile([128, R_tot], fp16, tag="mask")
            nc.sync.dma_start(mask_s[:], mask_d[:])
            ident = pp.tile([128, 128], fp16, tag="ident")
            masks.make_identity(nc, ident[:])

            qrr = [0]
            xTo = [pp.tile([128, PN], fp16, tag=f"xTo{i}", name=f"xTo{i}")
                   for i in range(2)]
            qs = pp.tile([128, G * 2 * D], fp16, tag="qs", name="qs")
            nc.sync.dma_start(xTo[0][:], xownT_d[:])

            for l in range(L_EFF):
                xT = xTo[l % 2]
                # ---- full k|v table, computed locally from replicated x ----
                if not NO_KVBUILD:
                    GH = 7  # groups per staged kvtab DMA (49 = 7x7)
                    for b in range(NCORES):
                        xfb = xp.tile([128, PN], fp16, tag="xfb")
                        nc.sync.dma_start(
                            xfb[:], xfull[l][b * D:(b + 1) * D, :])
                        for gh in range(G // GH):
                            stage = tp.tile([128, GH * 2 * D], fp16,
                                            tag="kvstage")
                            for g2 in range(GH):
                                g = gh * GH + g2
                                ps = psp.tile([128, 2 * D], fp32, tag="ps")
                                nc.tensor.matmul(
                                    ps[:], xfb[:, g * 128:(g + 1) * 128],
                                    wkv_s[:, l * 2 * D:(l + 1) * 2 * D])
                                dst = stage[:, g2 * 2 * D:(g2 + 1) * 2 * D]
                                if g2 % 2 == 0:
                                    nc.vector.tensor_copy(dst, ps[:])
                                else:
                                    nc.scalar.copy(dst, ps[:])
                            r0 = (b * G + gh * GH) * 128
                            nc.sync.dma_start(
                                kvtab[l][r0:r0 + GH * 128, :]
                                .rearrange("(g p) d -> p g d", p=128),
                                stage[:].rearrange("p (g d) -> p g d", g=GH))
                # ---- q|skip for own nodes (stays in SBUF) ----
                for g in range(G):
                    ps = psp.tile([128, 2 * D], fp32, tag="ps")
                    nc.tensor.matmul(
                        ps[:], xT[:, g * 128:(g + 1) * 128],
                        wqs_s[:, l * 2 * D:(l + 1) * 2 * D])
                    nc.vector.tensor_copy(
                        qs[:, g * 2 * D:(g + 1) * 2 * D], ps[:])

                # ---- edge phase: batches of K same-R groups ----
                xTn = xTo[(l + 1) % 2]
                for bi, (g0, K, R) in enumerate(batches):
                    if (COLL_FIRST and bi == 1 and l + 1 < L_EFF
                            and not NO_COLL):
                        # timing diagnostic: AG issued before its inputs are
                        # written, to measure overlap ceiling / gpsimd stall
                        nc.gpsimd.collective_compute(
                            "AllGather", ALU.bypass,
                            replica_groups=[list(range(NCORES))],
                            ins=[xnm[l + 1][:]], outs=[xfull[l + 1][:]])
                    KR = K * R
                    b0 = int(base[g0])
                    qsb = qs[:, g0 * 2 * D:(g0 + K) * 2 * D]
                    xn = sp.tile([128, K * D], fp16, tag="xn")
                    if R == 0 or NO_EDGE or GATHER_ONLY:
                        nc.vector.tensor_scalar_max(
                            xn[:].rearrange("p (k d) -> p k d", k=K),
                            qsb.rearrange("p (k t) -> p k t", k=K)
                            [:, :, D:2 * D], 0.0)
                    if R > 0 and not NO_EDGE:
                        kv = ep.tile([128, KR, 2 * D], fp16, tag="kv")
                        # SWDGE ring holds 1024 descriptors; one instruction
                        # must stay below that (7 rounds = 896)
                        CH = int(os.environ.get("KB_CH", "7"))
                        if GATHER_HALF == "bytes":
                            # timing diagnostic: 256B descriptors, same count
                            gdst = eq.tile([128, KR, D], fp16, tag="kvh")
                            gsrcs = (kvtab[l][:, 0:D],
                                     kvtab[l][SPLIT:NPAD, 0:D])
                            gkw = dict(elem_size=D, elem_step=2 * D)
                            dstep = 1
                        elif GATHER_HALF == "slots":
                            # timing diagnostic: half the descriptors, 512B
                            gdst = kv
                            gsrcs = (kvtab[l][:], kvtab[l][SPLIT:NPAD, :])
                            gkw = dict(elem_size=2 * D)
                            dstep = 2
                        else:
                            gdst = kv
                            gsrcs = (kvtab[l][:], kvtab[l][SPLIT:NPAD, :])
                            gkw = dict(elem_size=2 * D)
                            dstep = 1
                        for k in range(K):
                            ra, rb = int(RA[g0 + k]), int(RB[g0 + k])
                            bk = int(base[g0 + k])
                            for half, cnt, off in ((0, ra, 0), (1, rb, ra)):
                                for r0 in range(0, cnt, CH * dstep):
                                    rc = min(CH * dstep, cnt - r0)
                                    rcd = (rc + dstep - 1) // dstep
                                    s0 = k * R + off + r0
                                    nc.gpsimd.dma_gather(
                                        gdst[:, s0:s0 + rcd, :],
                                        gsrcs[half],
                                        kvidx_s[:, 8 * (bk + off + r0):
                                                8 * (bk + off + r0 + rcd)],
                                        num_idxs=128 * rcd,
                                        num_idxs_reg=128 * rcd,
                                        queue_num=qrr[0] % NQ,
                                        single_packet=SINGLE_PACKET, **gkw)
                                    qrr[0] += 1
                    if R > 0 and not NO_EDGE and not GATHER_ONLY:
                        qk = eq.tile([128, KR, D], fp16, tag="qkmsg")
                        nc.vector.tensor_mul(
                            qk[:].rearrange("p (k r) d -> p k r d", k=K),
                            kv[:, :, 0:D].rearrange(
                                "p (k r) d -> p k r d", k=K),
                            qsb.rearrange("p (k t) -> p k t", k=K)[:, :, 0:D]
                            .unsqueeze(2).broadcast_to([128, K, R, D]))
                        scm = sp.tile([128, KR * H], fp16, tag="scm")
                        if DOT_REDUCE:
                            # per-(edge,head) dot: single contiguous-X reduce
                            # (fp16 out matches the fp16 tree-add it replaces)
                            with nc.allow_low_precision("edge dot fp16"):
                                nc.vector.reduce_sum(
                                    scm[:].rearrange("p (kr h) -> p kr h",
                                                     h=H),
                                    qk[:].rearrange("p kr (h c) -> p (kr h) c",
                                                    h=H),
                                    axis=AX.X)
                        else:
                            qk4 = qk[:].rearrange("p kr (h c) -> p kr h c",
                                                  h=H)
                            w = C
                            while w > 2:
                                hw = w // 2
                                nc.vector.tensor_tensor(
                                    qk4[:, :, :, 0:hw], qk4[:, :, :, 0:hw],
                                    qk4[:, :, :, hw:w], op=ALU.add)
                                w = hw
                            nc.vector.tensor_tensor(
                                scm[:].rearrange("p (kr h) -> p kr h", h=H),
                                qk4[:, :, :, 0:1].rearrange(
                                    "p kr h c -> p kr (h c)"),
                                qk4[:, :, :, 1:2].rearrange(
                                    "p kr h c -> p kr (h c)"),
                                op=ALU.add)
                        nc.vector.tensor_tensor(
                            scm[:].rearrange("p (kr h) -> p kr h", h=H),
                            scm[:].rearrange("p (kr h) -> p kr h", h=H),
                            mask_s[:, b0:b0 + KR].unsqueeze(2)
                            .broadcast_to([128, KR, H]),
                            op=ALU.add)
                        mx = sp.tile([128, K * H], fp16, tag="mx")
                        nc.vector.reduce_max(
                            mx[:].rearrange("p (k h) -> p k h", k=K),
                            scm[:].rearrange("p (k r h) -> p k h r", k=K, h=H),
                            axis=AX.X)
                        pexp = sp.tile([128, KR * H], fp16, tag="pexp")
                        nc.vector.tensor_tensor(
                            pexp[:].rearrange("p (k r h) -> p k r h",
                                              k=K, h=H),
                            scm[:].rearrange("p (k r h) -> p k r h",
                                             k=K, h=H),
                            mx[:].rearrange("p (k h) -> p k h", k=K)
                            .unsqueeze(2).broadcast_to([128, K, R, H]),
                            op=ALU.subtract)
                        pexps = sp.tile([128, KR * H], fp16, tag="pexps")
                        nc.scalar.activation(pexps[:], pexp[:], ACTF.Exp)
                        pe128 = eq.tile([128, KR, D], fp16, tag="qkmsg")
                        if FUSE_EXP:
                            # msg = v * exp(score-max), exp broadcast over C
                            nc.vector.tensor_tensor(
                                pe128[:].rearrange("p kr (h c) -> p kr h c",
                                                   h=H),
                                kv[:, :, D:2 * D].rearrange(
                                    "p kr (h c) -> p kr h c", h=H),
                                pexps[:].rearrange("p (kr h) -> p kr h", h=H)
                                .unsqueeze(3).broadcast_to([128, KR, H, C]),
                                op=ALU.mult)
                        else:
                            nc.scalar.activation(
                                pe128[:].rearrange("p kr (h c) -> p kr h c",
                                                   h=H),
                                pexp[:].rearrange("p (kr h) -> p kr h", h=H)
                                .unsqueeze(3).broadcast_to([128, KR, H, C]),
                                ACTF.Exp)
                            nc.vector.tensor_tensor(
                                pe128[:], kv[:, :, D:2 * D], pe128[:],
                                op=ALU.mult)
                        z = sp.tile([128, K * H], fp32, tag="z")
                        nc.vector.reduce_sum(
                            z[:].rearrange("p (k h) -> p k h", k=K),
                            pexps[:].rearrange("p (k r h) -> p k h r",
                                               k=K, h=H),
                            axis=AX.X)
                        zi = sp.tile([128, K * H], fp16, tag="zi")
                        with nc.allow_low_precision("alpha normalizer fp16"):
                            nc.vector.reciprocal(zi[:], z[:])
                        pe4 = pe128[:].rearrange("p (k r) d -> p k r d", k=K)
                        n = R
                        while n > 1:
                            hw = n // 2
                            nc.vector.tensor_tensor(
                                pe4[:, :, 0:hw, :], pe4[:, :, 0:hw, :],
                                pe4[:, :, hw:2 * hw, :], op=ALU.add)
                            if n % 2 == 1:
                                nc.vector.tensor_tensor(
                                    pe4[:, :, 0:1, :], pe4[:, :, 0:1, :],
                                    pe4[:, :, 2 * hw:2 * hw + 1, :],
                                    op=ALU.add)
                            n = hw
                        xs = sp.tile([128, K * D], fp16, tag="xs")
                        nc.vector.tensor_tensor(
                            xs[:].rearrange("p (k h c) -> p k h c",
                                            k=K, h=H),
                            pe4[:, :, 0:1, :].rearrange(
                                "p k r (h c) -> p k (r h) c", h=H),
                            zi[:].rearrange("p (k h) -> p k h", k=K)
                            .unsqueeze(3).broadcast_to([128, K, H, C]),
                            op=ALU.mult)
                        nc.vector.tensor_tensor(
                            xs[:].rearrange("p (k d) -> p k d", k=K),
                            xs[:].rearrange("p (k d) -> p k d", k=K),
                            qsb.rearrange("p (k t) -> p k t", k=K)
                            [:, :, D:2 * D],
                            op=ALU.add)
                        nc.vector.tensor_scalar_max(xn[:], xs[:], 0.0)
                    # next layer needs own x feature-major (q|skip) and, for
                    # layers with a following AllGather, node-major in DRAM
                    psT = pspT.tile([128, K * 128], fp16, tag="psT")
                    for k in range(K):
                        nc.tensor.matmul(
                            psT[:, k * 128:(k + 1) * 128],
                            xn[:, k * D:(k + 1) * D], ident[:],
                            is_transpose=True)
                    nc.vector.tensor_copy(
                        xTn[:, g0 * 128:(g0 + K) * 128], psT[:])

                if l + 1 < L_EFF:
                    nc.sync.dma_start(xnm[l + 1][:], xTn[:])
                    if not NO_COLL and not COLL_FIRST:
                        nc.gpsimd.collective_compute(
                            "AllGather", ALU.bypass,
                            replica_groups=[list(range(NCORES))],
                            ins=[xnm[l + 1][:]], outs=[xfull[l + 1][:]])
                    tc.strict_bb_all_engine_barrier()

            # ---- final projection (own nodes) ----
            xT = xTo[L_EFF % 2]
            for g in range(G):
                ps = psp.tile([128, 2 * D], fp32, tag="ps")
                nc.tensor.matmul(ps[:, 0:OUT], xT[:, g * 128:(g + 1) * 128],
                                 wout_s[:])
                yo = tp.tile([128, OUT], fp32, tag="yo")
                nc.vector.tensor_copy(yo[:], ps[:, 0:OUT])
                nc.sync.dma_start(y_d[g * 128:(g + 1) * 128, :], yo[:])

    nc.compile()
    return nc


def _make_runner(nc, n_cores=NCORES):
    import jax
    from jax.sharding import Mesh, PartitionSpec
    from jax.experimental.shard_map import shard_map
    import concourse.mybir as mybir
    from concourse import bass2jax

    bass2jax.install_neuronx_cc_hook()
    partition_name = nc.partition_id_tensor.name if nc.partition_id_tensor else None
    in_names, out_names, out_avals, zero_outs = [], [], [], []
    for alloc in nc.m.functions[0].allocations:
        if not isinstance(alloc, mybir.MemoryLocationSet):
            continue
        name = alloc.memorylocations[0].name
        if alloc.kind == "ExternalInput":
            if name != partition_name:
                in_names.append(name)
        elif alloc.kind == "ExternalOutput":
            shape = tuple(alloc.tensor_shape)
            dtype = mybir.dt.np(alloc.dtype)
            out_names.append(name)
            out_avals.append(jax.core.ShapedArray(shape, dtype))
            zero_outs.append(np.zeros(shape, dtype))
    n_params = len(in_names)
    n_outs = len(out_avals)
    all_in_names = in_names + out_names + ([partition_name] if partition_name else [])
    donate = tuple(range(n_params, n_params + n_outs))

    def _body(*args):
        operands = list(args)
        if partition_name is not None:
            operands.append(bass2jax.partition_id_tensor())
        outs = bass2jax._bass_exec_p.bind(
            *operands, out_avals=tuple(out_avals), in_names=tuple(all_in_names),
            out_names=tuple(out_names), lowering_input_output_aliases=(),
            sim_require_finite=True, sim_require_nnan=True, nc=nc)
        return tuple(outs)

    devices = jax.devices()[:n_cores]
    mesh = Mesh(np.asarray(devices), ("core",))
    in_specs = (PartitionSpec("core"),) * (n_params + n_outs)
    out_specs = (PartitionSpec("core"),) * n_outs
    fn = jax.jit(shard_map(_body, mesh=mesh, in_specs=in_specs,
                           out_specs=out_specs, check_rep=False),
                 keep_unused=True)

    def run(in_maps, time_reps=0):
        concat_in = [
            np.concatenate([np.asarray(in_maps[c][nm]) for c in range(n_cores)], axis=0)
            for nm in in_names]
        concat_zeros = [np.zeros((n_cores * z.shape[0], *z.shape[1:]), z.dtype)
                        for z in zero_outs]
        args = [jax.device_put(a) for a in concat_in + concat_zeros]
        out = fn(*args)
        jax.block_until_ready(out)
        tmin = None
        if time_reps:
            ts = []
            for _ in range(time_reps):
                t0 = time.perf_counter()
                out = fn(*args)
                jax.block_until_ready(out)
                ts.append(time.perf_counter() - t0)
            tmin = min(ts)
        results = [
            {nm: np.asarray(out[i]).reshape(n_cores, *out_avals[i].shape)[c]
             for i, nm in enumerate(out_names)}
            for c in range(n_cores)]
        return results, tmin
    return run


def kernel(**inputs):
    x = np.asarray(inputs["x"], dtype=np.float32)
    edge_index = np.asarray(inputs["edge_index"])
    Wq = np.asarray(inputs["Wq"], dtype=np.float32)
    Wk = np.asarray(inputs["Wk"], dtype=np.float32)
    Wv = np.asarray(inputs["Wv"], dtype=np.float32)
    Wskip = np.asarray(inputs["Wskip"], dtype=np.float32)
    Wout = np.asarray(inputs["Wout"], dtype=np.float32)
    bout = np.asarray(inputs["bout"], dtype=np.float32)
    for b in ("bq", "bk", "bv", "bskip"):
        assert np.all(np.asarray(inputs[b]) == 0.0), f"{b} must be zero"

    if "prog" not in _cache:
        pi, RA, RB, R_tot, kv_idx, mask, batches = _host_prep(edge_index)
        nc = _build_program(RA, RB, R_tot, batches,
                            L_EFF=int(os.environ.get("KB_LAYERS", str(L))))
        run = _make_runner(nc)
        _cache["prog"] = (pi, R_tot, kv_idx, mask, run)
    pi, R_tot, kv_idx, mask, run = _cache["prog"]

    # q gets the 1/sqrt(C)=0.25 attention scale folded in
    wkv = np.transpose(np.concatenate([Wk, Wv], axis=2), (1, 0, 2)).reshape(
        D, L * 2 * D).astype(np.float16)
    wqs = np.transpose(np.concatenate([Wq * 0.25, Wskip], axis=2),
                       (1, 0, 2)).reshape(D, L * 2 * D).astype(np.float16)
    x0 = np.zeros((NPAD, D), dtype=np.float16)
    valid = pi >= 0
    x0[valid] = x[pi[valid]].astype(np.float16)
    x0T = np.ascontiguousarray(
        x0.reshape(NCORES, PN, D).transpose(0, 2, 1)).reshape(NCORES * D, PN)
    in_maps = []
    for c in range(NCORES):
        in_maps.append({
            "xownT": np.ascontiguousarray(x0[c * PN:(c + 1) * PN].T),
            "xfull0": x0T,
            "wkv": wkv,
            "wqs": wqs,
            "wout": Wout.astype(np.float16),
            "kvidx": kv_idx[c],
            "mask": mask[c],
        })
    results, tmin = run(in_maps, time_reps=int(os.environ.get("KB_TIME_REPS", "0")))
    globals()['_last_results'] = results
    globals()['_last_run'] = run
    globals()['_last_in_maps'] = in_maps
    if tmin is not None:
        globals()['_last_wall_us'] = tmin * 1e6
        print(f"kernel exec wall (min): {tmin*1e6:.0f} us")

    out = np.zeros((N, OUT), dtype=np.float32)
    for c in range(NCORES):
        yc = results[c]["y"]
        ids = pi[c * PN:(c + 1) * PN]
        m = ids >= 0
        out[ids[m]] = yc[m]
    return out + bout[None, :]
